# revision 20
# baseline (speedup 1.0000x reference)
"""Trainium2 Bass kernel for an AttnBlock (GroupNorm + spatial self-attention
+ projection + residual), distributed over 8 NeuronCores.

Sharding: core = (batch b, query-half h). b=4 batches x 2 halves = 8 cores.
Each core receives x[b] with its spatial columns rotated so that its own
query half occupies columns 0:2048 (attention is permutation-invariant over
key positions, so rotating the key/value axis consistently is exact).
No collectives needed: K/V are computed locally from the full (rotated) x[b].

Self-contained: hardcodes shapes (b=4, c=512, h=w=64).
"""
import numpy as np
import ml_dtypes

import bass_rust
import concourse.bass as bass
import concourse.mybir as mybir
from concourse import tile
from concourse.bass_utils import run_bass_kernel_spmd

f32 = mybir.dt.float32
bf16 = mybir.dt.bfloat16
AF = mybir.ActivationFunctionType

C = 512          # channels
N = 4096         # spatial positions (64*64)
M = 2048         # query positions per core (half)
P = 128          # partitions
CT = C // P      # 4 channel tiles
NT = N // P      # 32 n tiles
FB = 512         # free block (one PSUM bank of f32)
MB = M // FB     # 4 m-blocks per core
NG = 32          # groups
GSZ = C // NG    # 16 channels per group
EPS = 1e-6
RSCALE = 1.0 / np.sqrt(np.float32(C))   # attention scale
SSCALE = 1.0 / (GSZ * N)                # group-stat normalizer

_BF = ml_dtypes.bfloat16


def split_waits(nc, cap=1):
    """This walrus accepts one sync wait / one update per instruction; move
    extras onto adjacent same-engine NOPs (sequentially equivalent)."""
    for f in nc.m.functions:
        for bb in f.blocks:
            new_insts = []
            changed = False
            for inst in bb.instructions:
                si = inst.sync_info
                waits = list(si.on_wait) if si is not None else []
                ups = list(si.on_update) if si is not None else []
                if len(waits) > cap:
                    for ci in range(cap, len(waits), cap):
                        new_insts.append(mybir.InstNoOp(
                            name=f"{inst.name}-ws{ci}", engine=inst.engine,
                            ins=[], outs=[],
                            sync_info=bass_rust.SyncInfo(
                                on_wait=waits[ci:ci + cap], on_update=[])))
                    inst.sync_info = bass_rust.SyncInfo(
                        on_wait=waits[:cap], on_update=ups)
                    changed = True
                new_insts.append(inst)
                if len(ups) > 1:
                    inst.sync_info = bass_rust.SyncInfo(
                        on_wait=list(inst.sync_info.on_wait), on_update=ups[:1])
                    for ui in range(1, len(ups)):
                        new_insts.append(mybir.InstNoOp(
                            name=f"{inst.name}-us{ui}", engine=inst.engine,
                            ins=[], outs=[],
                            sync_info=bass_rust.SyncInfo(
                                on_wait=[], on_update=[ups[ui]])))
                    changed = True
            if changed:
                bb.instructions = new_insts


def build():
    nc = bass.Bass()

    xbf_e = nc.declare_dram_parameter("xbf", [C, N], bf16, isOutput=False)
    wq_e = nc.declare_dram_parameter("wq", [C, C], bf16, isOutput=False)
    wk_e = nc.declare_dram_parameter("wk", [C, C], bf16, isOutput=False)
    wv_e = nc.declare_dram_parameter("wv", [C, C], bf16, isOutput=False)
    wp_e = nc.declare_dram_parameter("wp", [C, C], bf16, isOutput=False)
    vecs_e = nc.declare_dram_parameter("vecs", [C, 4], f32, isOutput=False)
    gm_e = nc.declare_dram_parameter("gmask", [CT, P, NG], f32, isOutput=False)
    gmt_e = nc.declare_dram_parameter("gmaskT", [CT, NG + 1, P], f32, isOutput=False)
    ones_e = nc.declare_dram_parameter("ones", [P, P], bf16, isOutput=False)
    xres_e = nc.declare_dram_parameter("xres", [C, M], f32, isOutput=False)
    out_e = nc.declare_dram_parameter("out", [C, M], f32, isOutput=True)
    warm_e = nc.dram_tensor("warm_sink", [1, 4], f32)

    with tile.TileContext(nc) as tc:
        with (
            tc.tile_pool(name="const", bufs=1) as cp,
            tc.tile_pool(name="big", bufs=1) as bp,
            tc.tile_pool(name="small", bufs=1) as sp,
            tc.tile_pool(name="work", bufs=3) as wkp,
            tc.tile_pool(name="pmm", bufs=3, space="PSUM") as pmm,
            tc.tile_pool(name="pu", bufs=1, space="PSUM") as pu,
            tc.tile_pool(name="ps", bufs=1, space="PSUM") as psp,
        ):
            # ---- x in (bf16) first, chunked; stats overlap the DMA.
            # Constants go through gpsimd's queue so their issue cost doesn't
            # delay the critical xbf chunks on sync's queue. ----
            CH = 1024
            NCH = N // CH
            xbf_t = [bp.tile([P, N], bf16, tag=f"xbf{i}", name=f"xbf{i}") for i in range(CT)]
            for c in range(NCH):
                for i in range(CT):
                    eng = nc.sync if (c * CT + i) % 2 == 0 else nc.gpsimd
                    eng.dma_start(
                        xbf_t[i][:, c * CH:(c + 1) * CH],
                        xbf_e[i * P:(i + 1) * P, c * CH:(c + 1) * CH])

            vec_t = [cp.tile([P, 4], f32, tag=f"vec{i}", name=f"vec{i}") for i in range(CT)]
            gm_t = [cp.tile([P, NG], f32, tag=f"gm{i}", name=f"gm{i}") for i in range(CT)]
            gmt_t = [cp.tile([NG + 1, P], f32, tag=f"gmt{i}", name=f"gmt{i}") for i in range(CT)]
            for i in range(CT):
                sl = slice(i * P, (i + 1) * P)
                nc.sync.dma_start(vec_t[i][:], vecs_e[sl, :])
                nc.sync.dma_start(gm_t[i][:], gm_e[i, :, :])
                nc.sync.dma_start(gmt_t[i][:], gmt_e[i, :, :])
            ones_t = cp.tile([P, P], bf16, tag="ones", name="ones")
            nc.gpsimd.dma_start(ones_t[:], ones_e[:])
            wq_t = [cp.tile([P, C], bf16, tag=f"wq{i}", name=f"wq{i}") for i in range(CT)]
            wk_t = [cp.tile([P, C], bf16, tag=f"wk{i}", name=f"wk{i}") for i in range(CT)]
            wv_t = [cp.tile([P, C], bf16, tag=f"wv{i}", name=f"wv{i}") for i in range(CT)]
            wp_t = [cp.tile([P, C], bf16, tag=f"wp{i}", name=f"wp{i}") for i in range(CT)]
            for i in range(CT):
                nc.gpsimd.dma_start(wq_t[i][:], wq_e[i * P:(i + 1) * P, :])
                nc.gpsimd.dma_start(wk_t[i][:], wk_e[i * P:(i + 1) * P, :])
                nc.gpsimd.dma_start(wv_t[i][:], wv_e[i * P:(i + 1) * P, :])
                nc.gpsimd.dma_start(wp_t[i][:], wp_e[i * P:(i + 1) * P, :])
            bq_t = [vec_t[i][:, 0:1] for i in range(CT)]
            bk_t = [vec_t[i][:, 1:2] for i in range(CT)]
            gam_t = [vec_t[i][:, 2:3] for i in range(CT)]
            bet_t = [vec_t[i][:, 3:4] for i in range(CT)]

            # ---- group norm stats (chunked partials; the gather matmul
            # accumulates per (tile, chunk) so it only waits on that chunk) ----
            hn_t = [bp.tile([P, N], bf16, tag=f"hn{i}", name=f"hn{i}") for i in range(CT)]
            st8_t = [sp.tile([P, NCH, 2], f32, tag=f"st8{i}", name=f"st8{i}")
                     for i in range(CT)]
            for c in range(NCH):
                for i in range(CT):
                    csl = slice(c * CH, (c + 1) * CH)
                    # per-channel sum (DVE) and sum of squares (ACT accum); the
                    # ACT pass writes squares into hn as scratch (overwritten by
                    # the affine apply later).
                    nc.vector.tensor_reduce(
                        st8_t[i][:, c, 0:1], xbf_t[i][:, csl],
                        axis=mybir.AxisListType.X, op=mybir.AluOpType.add)
                    nc.scalar.activation(
                        hn_t[i][:, csl], xbf_t[i][:, csl], AF.Square,
                        accum_out=st8_t[i][:, c, 1:2])
            gps = psp.tile([NG, 2], f32, tag="s", name="s")
            for c in range(NCH):
                for i in range(CT):
                    nc.tensor.matmul(
                        gps[:], gm_t[i][:], st8_t[i][:, c, :],
                        start=(c == 0 and i == 0),
                        stop=(c == NCH - 1 and i == CT - 1),
                        skip_group_check=True)
            # PE warm-up: HAM throttles PE to 1.2 GHz after idle; these dummy
            # matmuls fill the scalar-chain window so the projection matmuls
            # start at full clock. Token DMA keeps the chain live.
            WARMUP = 0
            if WARMUP:
                wps = pmm.tile([P, P], f32, tag="mm", name="warmps")
                for _ in range(WARMUP):
                    nc.tensor.matmul(wps[:], ones_t[:], ones_t[:],
                                     start=True, stop=True)
                wsb = sp.tile([1, 4], f32, tag="wsb", name="wsb")
                nc.vector.tensor_copy(wsb[:], wps[0:1, 0:4])
                nc.sync.dma_start(warm_e[:], wsb[:])
            # mean, rstd; gstat[:,1] transiently holds msq, then rstd
            gstat = sp.tile([NG, 2], f32, tag="gstat", name="gstat")   # [mean, rstd]
            mean = gstat[:, 0:1]
            nc.vector.tensor_scalar_mul(gstat[:, 0:2], gps[:, 0:2], SSCALE)
            m2 = sp.tile([NG, 1], f32, tag="m2", name="m2")
            nc.vector.tensor_mul(m2[:], mean, mean)
            varp = sp.tile([NG, 1], f32, tag="varp", name="varp")
            nc.vector.tensor_sub(varp[:], gstat[:, 1:2], m2[:])
            nc.vector.tensor_scalar_add(varp[:], varp[:], EPS)
            std = sp.tile([NG, 1], f32, tag="std", name="std")
            nc.scalar.activation(std[:], varp[:], AF.Sqrt)
            nc.vector.reciprocal(gstat[:, 1:2], std[:])

            # rhs33 = [[-mean*rstd, rstd]; [1, 0]]: with the gamma-scaled,
            # beta-extended maskT as lhsT, one matmul per tile produces
            # ex = [bias, scale] per channel (bias = beta - mean*gamma*rstd,
            # scale = gamma*rstd).
            rhs33 = sp.tile([NG + 1, 2], f32, tag="rhs33", name="rhs33")
            nc.gpsimd.memset(rhs33[NG:NG + 1, 0:1], 1.0)
            nc.gpsimd.memset(rhs33[NG:NG + 1, 1:2], 0.0)
            mr = sp.tile([NG, 1], f32, tag="mr", name="mr")
            nc.vector.tensor_mul(mr[:], gstat[:, 0:1], gstat[:, 1:2])
            nc.vector.tensor_scalar_mul(rhs33[0:NG, 0:1], mr[:], -1.0)
            nc.vector.tensor_copy(rhs33[0:NG, 1:2], gstat[:, 1:2])
            ab_t = []
            for i in range(CT):
                eps_p = pmm.tile([P, 2], f32, tag="mm", name="mm")
                nc.tensor.matmul(eps_p[:], gmt_t[i][:], rhs33[:],
                                 start=True, stop=True)
                ex = sp.tile([P, 2], f32, tag=f"ex{i}", name=f"ex{i}")
                nc.vector.tensor_copy(ex[:], eps_p[:])
                ab_t.append(ex)
            # chunked apply (512-wide) so downstream matmuls start early
            for c in range(N // FB):
                for i in range(CT):
                    csl = slice(c * FB, (c + 1) * FB)
                    nc.scalar.activation(
                        hn_t[i][:, csl], xbf_t[i][:, csl], AF.Identity,
                        bias=ab_t[i][:, 0:1], scale=ab_t[i][:, 1:2])

            # ---- projections ----
            q_t = [bp.tile([P, M], bf16, tag=f"q{i}", name=f"q{i}") for i in range(CT)]
            k_t = [bp.tile([P, N], bf16, tag=f"k{i}", name=f"k{i}") for i in range(CT)]
            vt_t = bp.tile([P, NT * C], bf16, tag="vt", name="vt")  # [n-tile stack | c]

            for ot in range(CT):
                for b in range(MB):
                    ps = pmm.tile([P, FB], f32, tag="mm", name="mm")
                    for kt in range(CT):
                        nc.tensor.matmul(
                            ps[:], wq_t[kt][:, ot * P:(ot + 1) * P],
                            hn_t[kt][:, b * FB:(b + 1) * FB],
                            start=(kt == 0), stop=(kt == CT - 1))
                    nc.vector.tensor_scalar_add(
                        q_t[ot][:, b * FB:(b + 1) * FB], ps[:], bq_t[ot])
            for ot in range(CT):
                for b in range(N // FB):
                    ps = pmm.tile([P, FB], f32, tag="mm", name="mm")
                    for kt in range(CT):
                        nc.tensor.matmul(
                            ps[:], wk_t[kt][:, ot * P:(ot + 1) * P],
                            hn_t[kt][:, b * FB:(b + 1) * FB],
                            start=(kt == 0), stop=(kt == CT - 1))
                    nc.vector.tensor_scalar_add(
                        k_t[ot][:, b * FB:(b + 1) * FB], ps[:], bk_t[ot])
            for nt in range(NT):
                ps = pmm.tile([P, C], f32, tag="mm", name="mm")
                for kt in range(CT):
                    nc.tensor.matmul(
                        ps[:], hn_t[kt][:, nt * P:(nt + 1) * P], wv_t[kt][:],
                        start=(kt == 0), stop=(kt == CT - 1))
                nc.vector.tensor_copy(vt_t[:, nt * C:(nt + 1) * C], ps[:])

            # ---- attention (per m-block) ----
            # Software-pipelined: exp consumption lags the score matmuls by
            # LAG n-tiles, and the previous m-block's tail (reciprocal,
            # normalize, projection, residual, store) is emitted a few
            # n-tiles into the next block so PE never waits on DVE.
            LAG = 4   # u-matmul consumption lag (n-tiles)
            SLAG = 1  # s-matmul lag: early so the reciprocal overlaps the
                      # final u-matmuls instead of serializing after them

            def consume_u(b, j, e_sb, u_ps):
                for ct in range(CT):
                    nc.tensor.matmul(
                        u_ps[ct][:],
                        vt_t[:, j * C + ct * P: j * C + (ct + 1) * P],
                        e_sb[:],
                        start=(j == 0), stop=(j == NT - 1),
                        skip_group_check=True)

            def consume_s(b, j, e_sb, s_ps):
                nc.tensor.matmul(
                    s_ps[:], ones_t[:], e_sb[:],
                    start=(j == 0), stop=(j == NT - 1), skip_group_check=True)

            def emit_tail(b, u_ps, s_ps, nsplit=1):
                # nsplit>1 shortens the serial reciprocal->normalize->project
                # chain; used for the final block where nothing hides it.
                HB = FB // nsplit
                for hb in range(nsplit):
                    hsl = slice(hb * HB, (hb + 1) * HB)
                    msl = slice(b * FB + hb * HB, b * FB + (hb + 1) * HB)
                    r_sb = wkp.tile([P, HB], f32, tag="r", name="r")
                    nc.vector.reciprocal(r_sb[:], s_ps[:, hsl])
                    u_sb = [wkp.tile([P, HB], bf16, tag=f"usb{ct}",
                                     name=f"usb{ct}") for ct in range(CT)]
                    for ct in range(CT):
                        nc.vector.tensor_mul(u_sb[ct][:], u_ps[ct][:, hsl],
                                             r_sb[:])
                    for ot in range(CT):
                        pp = pmm.tile([P, HB], f32, tag="mm", name="mm")
                        for kt in range(CT):
                            nc.tensor.matmul(
                                pp[:], wp_t[kt][:, ot * P:(ot + 1) * P],
                                u_sb[kt][:],
                                start=(kt == 0), stop=(kt == CT - 1))
                        xr = wkp.tile([P, HB], f32, tag="xr", name="xr")
                        nc.sync.dma_start(
                            xr[:], xres_e[ot * P:(ot + 1) * P, msl])
                        o_sb = wkp.tile([P, HB], f32, tag="o", name="o")
                        nc.vector.tensor_add(o_sb[:], pp[:], xr[:])
                        nc.sync.dma_start(
                            out_e[ot * P:(ot + 1) * P, msl], o_sb[:])

            prev_tail = None
            for b in range(MB):
                msl = slice(b * FB, (b + 1) * FB)
                u_ps = [pu.tile([P, FB], f32, tag=f"u{ct}", name=f"u{ct}")
                        for ct in range(CT)]
                s_ps = psp.tile([P, FB], f32, tag="s", name="s")
                es = []
                for nt in range(NT):
                    sc = pmm.tile([P, FB], f32, tag="mm", name="mm")
                    for kt in range(CT):
                        nc.tensor.matmul(
                            sc[:], k_t[kt][:, nt * P:(nt + 1) * P],
                            q_t[kt][:, msl],
                            start=(kt == 0), stop=(kt == CT - 1))
                    if nt >= SLAG:
                        consume_s(b, nt - SLAG, es[nt - SLAG], s_ps)
                    e_sb = wkp.tile([P, FB], bf16, tag="e", name="e", bufs=LAG + 2)
                    nc.scalar.activation(e_sb[:], sc[:], AF.Exp, scale=RSCALE)
                    es.append(e_sb)
                    if nt == LAG - 1 and prev_tail is not None:
                        emit_tail(*prev_tail)
                        prev_tail = None
                    if nt >= LAG:
                        consume_u(b, nt - LAG, es[nt - LAG], u_ps)
                for j in range(NT - SLAG, NT):
                    consume_s(b, j, es[j], s_ps)
                for j in range(NT - LAG, NT):
                    consume_u(b, j, es[j], u_ps)
                prev_tail = (b, u_ps, s_ps)
            emit_tail(*prev_tail)

    split_waits(nc)
    return nc


_NC_CACHE = None


def _get_nc():
    global _NC_CACHE
    if _NC_CACHE is None:
        _NC_CACHE = build()
    return _NC_CACHE


def _prep_inputs(x, gamma, beta, Wq, bq, Wk, bk, Wv, bv, Wp, bp):
    """Build the 8 per-core input maps from full inputs."""
    B = x.shape[0]
    xf = np.ascontiguousarray(x.reshape(B, C, N)).astype(np.float32)
    bp_eff = (bp + Wp @ bv).astype(np.float32)

    gmask = np.zeros((CT, P, NG), np.float32)
    gmaskT = np.zeros((CT, NG + 1, P), np.float32)
    gf = gamma.astype(np.float32)
    bf = beta.astype(np.float32)
    for t in range(CT):
        for p in range(P):
            ch = t * P + p
            g = ch // GSZ
            gmask[t, p, g] = 1.0
            gmaskT[t, g, p] = gf[ch]
            gmaskT[t, NG, p] = bf[ch]

    shared = {
        "wq": np.ascontiguousarray(Wq.T).astype(_BF),
        "wk": np.ascontiguousarray(Wk.T).astype(_BF),
        "wv": np.ascontiguousarray(Wv.T).astype(_BF),
        "wp": np.ascontiguousarray(Wp.T).astype(_BF),
        "vecs": np.stack([bq, bk, gamma, beta], axis=1).astype(np.float32),
        "gmask": gmask,
        "gmaskT": gmaskT,
        "ones": np.ones((P, P), _BF),
    }
    in_maps = []
    for core in range(2 * B):
        b, h = divmod(core, 2)
        xb = xf[b]
        if h == 0:
            xp = xb
        else:
            xp = np.concatenate([xb[:, M:], xb[:, :M]], axis=1)
        m = dict(shared)
        m["xbf"] = np.ascontiguousarray(xp).astype(_BF)
        m["xres"] = np.ascontiguousarray(xp[:, :M]) + bp_eff[:, None]
        in_maps.append(m)
    return in_maps


def run(inputs, trace=False, **kw):
    x = np.asarray(inputs["x"], np.float32)
    B = x.shape[0]
    in_maps = _prep_inputs(**{k: np.asarray(v) for k, v in inputs.items()})
    nc = _get_nc()
    res = run_bass_kernel_spmd(nc, in_maps, core_ids=list(range(8)),
                               trace=trace, **kw)
    out = np.empty((B, C, N), np.float32)
    for core in range(2 * B):
        b, h = divmod(core, 2)
        out[b][:, h * M:(h + 1) * M] = res.results[core]["out"]
    return out.reshape(x.shape), res


def kernel(**inputs):
    out, _ = run(inputs, trace=False)
    return out


# revision 21
# speedup vs baseline: 1.0067x; 1.0067x over previous
"""Trainium2 Bass kernel for an AttnBlock (GroupNorm + spatial self-attention
+ projection + residual), distributed over 8 NeuronCores.

Sharding: core = (batch b, query-half h). b=4 batches x 2 halves = 8 cores.
Each core receives x[b] with its spatial columns rotated so that its own
query half occupies columns 0:2048 (attention is permutation-invariant over
key positions, so rotating the key/value axis consistently is exact).
No collectives needed: K/V are computed locally from the full (rotated) x[b].

Self-contained: hardcodes shapes (b=4, c=512, h=w=64).
"""
import numpy as np
import ml_dtypes

import bass_rust
import concourse.bass as bass
import concourse.mybir as mybir
from concourse import tile
from concourse.bass_utils import run_bass_kernel_spmd

f32 = mybir.dt.float32
bf16 = mybir.dt.bfloat16
AF = mybir.ActivationFunctionType

C = 512          # channels
N = 4096         # spatial positions (64*64)
M = 2048         # query positions per core (half)
P = 128          # partitions
CT = C // P      # 4 channel tiles
NT = N // P      # 32 n tiles
FB = 512         # free block (one PSUM bank of f32)
MB = M // FB     # 4 m-blocks per core
NG = 32          # groups
GSZ = C // NG    # 16 channels per group
EPS = 1e-6
RSCALE = 1.0 / np.sqrt(np.float32(C))   # attention scale
SSCALE = 1.0 / (GSZ * N)                # group-stat normalizer

_BF = ml_dtypes.bfloat16


def split_waits(nc, cap=1):
    """This walrus accepts one sync wait / one update per instruction; move
    extras onto adjacent same-engine NOPs (sequentially equivalent)."""
    for f in nc.m.functions:
        for bb in f.blocks:
            new_insts = []
            changed = False
            for inst in bb.instructions:
                si = inst.sync_info
                waits = list(si.on_wait) if si is not None else []
                ups = list(si.on_update) if si is not None else []
                if len(waits) > cap:
                    for ci in range(cap, len(waits), cap):
                        new_insts.append(mybir.InstNoOp(
                            name=f"{inst.name}-ws{ci}", engine=inst.engine,
                            ins=[], outs=[],
                            sync_info=bass_rust.SyncInfo(
                                on_wait=waits[ci:ci + cap], on_update=[])))
                    inst.sync_info = bass_rust.SyncInfo(
                        on_wait=waits[:cap], on_update=ups)
                    changed = True
                new_insts.append(inst)
                if len(ups) > 1:
                    inst.sync_info = bass_rust.SyncInfo(
                        on_wait=list(inst.sync_info.on_wait), on_update=ups[:1])
                    for ui in range(1, len(ups)):
                        new_insts.append(mybir.InstNoOp(
                            name=f"{inst.name}-us{ui}", engine=inst.engine,
                            ins=[], outs=[],
                            sync_info=bass_rust.SyncInfo(
                                on_wait=[], on_update=[ups[ui]])))
                    changed = True
            if changed:
                bb.instructions = new_insts


def build():
    nc = bass.Bass()

    xbf_e = nc.declare_dram_parameter("xbf", [C, N], bf16, isOutput=False)
    wq_e = nc.declare_dram_parameter("wq", [C, C], bf16, isOutput=False)
    wk_e = nc.declare_dram_parameter("wk", [C, C], bf16, isOutput=False)
    wv_e = nc.declare_dram_parameter("wv", [C, C], bf16, isOutput=False)
    wp_e = nc.declare_dram_parameter("wp", [C, C], bf16, isOutput=False)
    vecs_e = nc.declare_dram_parameter("vecs", [C, 4], f32, isOutput=False)
    gm_e = nc.declare_dram_parameter("gmask", [CT, P, NG], f32, isOutput=False)
    gmt_e = nc.declare_dram_parameter("gmaskT", [CT, NG + 1, P], f32, isOutput=False)
    ones_e = nc.declare_dram_parameter("ones", [P, P], bf16, isOutput=False)
    xres_e = nc.declare_dram_parameter("xres", [C, M], f32, isOutput=False)
    out_e = nc.declare_dram_parameter("out", [C, M], f32, isOutput=True)
    warm_e = nc.dram_tensor("warm_sink", [1, 4], f32)

    with tile.TileContext(nc) as tc:
        with (
            tc.tile_pool(name="const", bufs=1) as cp,
            tc.tile_pool(name="big", bufs=1) as bp,
            tc.tile_pool(name="small", bufs=1) as sp,
            tc.tile_pool(name="work", bufs=3) as wkp,
            tc.tile_pool(name="pmm", bufs=3, space="PSUM") as pmm,
            tc.tile_pool(name="pu", bufs=1, space="PSUM") as pu,
            tc.tile_pool(name="ps", bufs=1, space="PSUM") as psp,
        ):
            # ---- x in (bf16) first, chunked; stats overlap the DMA.
            # Constants go through gpsimd's queue so their issue cost doesn't
            # delay the critical xbf chunks on sync's queue. ----
            xbf_t = [bp.tile([P, N], bf16, tag=f"xbf{i}", name=f"xbf{i}") for i in range(CT)]
            for i in range(CT):
                eng = nc.sync if i % 2 == 0 else nc.gpsimd
                eng.dma_start(xbf_t[i][:], xbf_e[i * P:(i + 1) * P, :])

            vec_t = [cp.tile([P, 4], f32, tag=f"vec{i}", name=f"vec{i}") for i in range(CT)]
            gm_t = [cp.tile([P, NG], f32, tag=f"gm{i}", name=f"gm{i}") for i in range(CT)]
            gmt_t = [cp.tile([NG + 1, P], f32, tag=f"gmt{i}", name=f"gmt{i}") for i in range(CT)]
            for i in range(CT):
                sl = slice(i * P, (i + 1) * P)
                nc.sync.dma_start(vec_t[i][:], vecs_e[sl, :])
                nc.sync.dma_start(gm_t[i][:], gm_e[i, :, :])
                nc.sync.dma_start(gmt_t[i][:], gmt_e[i, :, :])
            ones_t = cp.tile([P, P], bf16, tag="ones", name="ones")
            nc.gpsimd.dma_start(ones_t[:], ones_e[:])
            wq_t = [cp.tile([P, C], bf16, tag=f"wq{i}", name=f"wq{i}") for i in range(CT)]
            wk_t = [cp.tile([P, C], bf16, tag=f"wk{i}", name=f"wk{i}") for i in range(CT)]
            wv_t = [cp.tile([P, C], bf16, tag=f"wv{i}", name=f"wv{i}") for i in range(CT)]
            wp_t = [cp.tile([P, C], bf16, tag=f"wp{i}", name=f"wp{i}") for i in range(CT)]
            for i in range(CT):
                nc.gpsimd.dma_start(wq_t[i][:], wq_e[i * P:(i + 1) * P, :])
                nc.gpsimd.dma_start(wk_t[i][:], wk_e[i * P:(i + 1) * P, :])
                nc.gpsimd.dma_start(wv_t[i][:], wv_e[i * P:(i + 1) * P, :])
                nc.gpsimd.dma_start(wp_t[i][:], wp_e[i * P:(i + 1) * P, :])
            bq_t = [vec_t[i][:, 0:1] for i in range(CT)]
            bk_t = [vec_t[i][:, 1:2] for i in range(CT)]
            gam_t = [vec_t[i][:, 2:3] for i in range(CT)]
            bet_t = [vec_t[i][:, 3:4] for i in range(CT)]

            # ---- group norm stats (whole-tile ops; the gather matmul
            # accumulates per tile so the 4-stage DMA->stats pipe overlaps) ----
            hn_t = [bp.tile([P, N], bf16, tag=f"hn{i}", name=f"hn{i}") for i in range(CT)]
            st2_t = [sp.tile([P, 2], f32, tag=f"st2{i}", name=f"st2{i}")
                     for i in range(CT)]
            for i in range(CT):
                # per-channel sum (DVE) and sum of squares (ACT accum); the
                # ACT pass writes squares into hn as scratch (overwritten by
                # the affine apply later).
                nc.vector.tensor_reduce(
                    st2_t[i][:, 0:1], xbf_t[i][:],
                    axis=mybir.AxisListType.X, op=mybir.AluOpType.add)
                nc.scalar.activation(
                    hn_t[i][:], xbf_t[i][:], AF.Square,
                    accum_out=st2_t[i][:, 1:2])
            gps = psp.tile([NG, 2], f32, tag="s", name="s")
            for i in range(CT):
                nc.tensor.matmul(
                    gps[:], gm_t[i][:], st2_t[i][:],
                    start=(i == 0), stop=(i == CT - 1),
                    skip_group_check=True)
            # PE warm-up: HAM throttles PE to 1.2 GHz after idle; these dummy
            # matmuls fill the scalar-chain window so the projection matmuls
            # start at full clock. Token DMA keeps the chain live.
            WARMUP = 0
            if WARMUP:
                wps = pmm.tile([P, P], f32, tag="mm", name="warmps")
                for _ in range(WARMUP):
                    nc.tensor.matmul(wps[:], ones_t[:], ones_t[:],
                                     start=True, stop=True)
                wsb = sp.tile([1, 4], f32, tag="wsb", name="wsb")
                nc.vector.tensor_copy(wsb[:], wps[0:1, 0:4])
                nc.sync.dma_start(warm_e[:], wsb[:])
            # mean, rstd; gstat[:,1] transiently holds msq, then rstd
            gstat = sp.tile([NG, 2], f32, tag="gstat", name="gstat")   # [mean, rstd]
            mean = gstat[:, 0:1]
            nc.vector.tensor_scalar_mul(gstat[:, 0:2], gps[:, 0:2], SSCALE)
            m2 = sp.tile([NG, 1], f32, tag="m2", name="m2")
            nc.vector.tensor_mul(m2[:], mean, mean)
            varp = sp.tile([NG, 1], f32, tag="varp", name="varp")
            nc.vector.tensor_sub(varp[:], gstat[:, 1:2], m2[:])
            nc.vector.tensor_scalar_add(varp[:], varp[:], EPS)
            std = sp.tile([NG, 1], f32, tag="std", name="std")
            nc.scalar.activation(std[:], varp[:], AF.Sqrt)
            nc.vector.reciprocal(gstat[:, 1:2], std[:])

            # rhs33 = [[-mean*rstd, rstd]; [1, 0]]: with the gamma-scaled,
            # beta-extended maskT as lhsT, one matmul per tile produces
            # ex = [bias, scale] per channel (bias = beta - mean*gamma*rstd,
            # scale = gamma*rstd).
            rhs33 = sp.tile([NG + 1, 2], f32, tag="rhs33", name="rhs33")
            nc.gpsimd.memset(rhs33[NG:NG + 1, 0:1], 1.0)
            nc.gpsimd.memset(rhs33[NG:NG + 1, 1:2], 0.0)
            mr = sp.tile([NG, 1], f32, tag="mr", name="mr")
            nc.vector.tensor_mul(mr[:], gstat[:, 0:1], gstat[:, 1:2])
            nc.vector.tensor_scalar_mul(rhs33[0:NG, 0:1], mr[:], -1.0)
            nc.vector.tensor_copy(rhs33[0:NG, 1:2], gstat[:, 1:2])
            ab_t = []
            for i in range(CT):
                eps_p = pmm.tile([P, 2], f32, tag="mm", name="mm")
                nc.tensor.matmul(eps_p[:], gmt_t[i][:], rhs33[:],
                                 start=True, stop=True)
                ex = sp.tile([P, 2], f32, tag=f"ex{i}", name=f"ex{i}")
                nc.vector.tensor_copy(ex[:], eps_p[:])
                ab_t.append(ex)
            # chunked apply (512-wide) so downstream matmuls start early
            for c in range(N // FB):
                for i in range(CT):
                    csl = slice(c * FB, (c + 1) * FB)
                    nc.scalar.activation(
                        hn_t[i][:, csl], xbf_t[i][:, csl], AF.Identity,
                        bias=ab_t[i][:, 0:1], scale=ab_t[i][:, 1:2])

            # ---- projections ----
            q_t = [bp.tile([P, M], bf16, tag=f"q{i}", name=f"q{i}") for i in range(CT)]
            k_t = [bp.tile([P, N], bf16, tag=f"k{i}", name=f"k{i}") for i in range(CT)]
            vt_t = bp.tile([P, NT * C], bf16, tag="vt", name="vt")  # [n-tile stack | c]

            for ot in range(CT):
                for b in range(MB):
                    ps = pmm.tile([P, FB], f32, tag="mm", name="mm")
                    for kt in range(CT):
                        nc.tensor.matmul(
                            ps[:], wq_t[kt][:, ot * P:(ot + 1) * P],
                            hn_t[kt][:, b * FB:(b + 1) * FB],
                            start=(kt == 0), stop=(kt == CT - 1))
                    nc.vector.tensor_scalar_add(
                        q_t[ot][:, b * FB:(b + 1) * FB], ps[:], bq_t[ot])
            for ot in range(CT):
                for b in range(N // FB):
                    ps = pmm.tile([P, FB], f32, tag="mm", name="mm")
                    for kt in range(CT):
                        nc.tensor.matmul(
                            ps[:], wk_t[kt][:, ot * P:(ot + 1) * P],
                            hn_t[kt][:, b * FB:(b + 1) * FB],
                            start=(kt == 0), stop=(kt == CT - 1))
                    nc.vector.tensor_scalar_add(
                        k_t[ot][:, b * FB:(b + 1) * FB], ps[:], bk_t[ot])
            for nt in range(NT):
                ps = pmm.tile([P, C], f32, tag="mm", name="mm")
                for kt in range(CT):
                    nc.tensor.matmul(
                        ps[:], hn_t[kt][:, nt * P:(nt + 1) * P], wv_t[kt][:],
                        start=(kt == 0), stop=(kt == CT - 1))
                nc.vector.tensor_copy(vt_t[:, nt * C:(nt + 1) * C], ps[:])

            # ---- attention (per m-block) ----
            # Software-pipelined: exp consumption lags the score matmuls by
            # LAG n-tiles, and the previous m-block's tail (reciprocal,
            # normalize, projection, residual, store) is emitted a few
            # n-tiles into the next block so PE never waits on DVE.
            LAG = 4   # u-matmul consumption lag (n-tiles)
            SLAG = 1  # s-matmul lag: early so the reciprocal overlaps the
                      # final u-matmuls instead of serializing after them

            def consume_u(b, j, e_sb, u_ps):
                for ct in range(CT):
                    nc.tensor.matmul(
                        u_ps[ct][:],
                        vt_t[:, j * C + ct * P: j * C + (ct + 1) * P],
                        e_sb[:],
                        start=(j == 0), stop=(j == NT - 1),
                        skip_group_check=True)

            def consume_s(b, j, e_sb, s_ps):
                nc.tensor.matmul(
                    s_ps[:], ones_t[:], e_sb[:],
                    start=(j == 0), stop=(j == NT - 1), skip_group_check=True)

            def emit_tail(b, u_ps, s_ps, nsplit=1):
                # nsplit>1 shortens the serial reciprocal->normalize->project
                # chain; used for the final block where nothing hides it.
                HB = FB // nsplit
                for hb in range(nsplit):
                    hsl = slice(hb * HB, (hb + 1) * HB)
                    msl = slice(b * FB + hb * HB, b * FB + (hb + 1) * HB)
                    r_sb = wkp.tile([P, HB], f32, tag="r", name="r")
                    nc.vector.reciprocal(r_sb[:], s_ps[:, hsl])
                    u_sb = [wkp.tile([P, HB], bf16, tag=f"usb{ct}",
                                     name=f"usb{ct}") for ct in range(CT)]
                    for ct in range(CT):
                        nc.vector.tensor_mul(u_sb[ct][:], u_ps[ct][:, hsl],
                                             r_sb[:])
                    for ot in range(CT):
                        pp = pmm.tile([P, HB], f32, tag="mm", name="mm")
                        for kt in range(CT):
                            nc.tensor.matmul(
                                pp[:], wp_t[kt][:, ot * P:(ot + 1) * P],
                                u_sb[kt][:],
                                start=(kt == 0), stop=(kt == CT - 1))
                        xr = wkp.tile([P, HB], f32, tag="xr", name="xr")
                        nc.sync.dma_start(
                            xr[:], xres_e[ot * P:(ot + 1) * P, msl])
                        o_sb = wkp.tile([P, HB], f32, tag="o", name="o")
                        nc.vector.tensor_add(o_sb[:], pp[:], xr[:])
                        nc.sync.dma_start(
                            out_e[ot * P:(ot + 1) * P, msl], o_sb[:])

            prev_tail = None
            for b in range(MB):
                msl = slice(b * FB, (b + 1) * FB)
                u_ps = [pu.tile([P, FB], f32, tag=f"u{ct}", name=f"u{ct}")
                        for ct in range(CT)]
                s_ps = psp.tile([P, FB], f32, tag="s", name="s")
                es = []
                for nt in range(NT):
                    sc = pmm.tile([P, FB], f32, tag="mm", name="mm")
                    for kt in range(CT):
                        nc.tensor.matmul(
                            sc[:], k_t[kt][:, nt * P:(nt + 1) * P],
                            q_t[kt][:, msl],
                            start=(kt == 0), stop=(kt == CT - 1))
                    if nt >= SLAG:
                        consume_s(b, nt - SLAG, es[nt - SLAG], s_ps)
                    e_sb = wkp.tile([P, FB], bf16, tag="e", name="e", bufs=LAG + 2)
                    nc.scalar.activation(e_sb[:], sc[:], AF.Exp, scale=RSCALE)
                    es.append(e_sb)
                    if nt == LAG - 1 and prev_tail is not None:
                        emit_tail(*prev_tail)
                        prev_tail = None
                    if nt >= LAG:
                        consume_u(b, nt - LAG, es[nt - LAG], u_ps)
                for j in range(NT - SLAG, NT):
                    consume_s(b, j, es[j], s_ps)
                for j in range(NT - LAG, NT):
                    consume_u(b, j, es[j], u_ps)
                prev_tail = (b, u_ps, s_ps)
            emit_tail(*prev_tail)

    split_waits(nc)
    return nc


_NC_CACHE = None


def _get_nc():
    global _NC_CACHE
    if _NC_CACHE is None:
        _NC_CACHE = build()
    return _NC_CACHE


def _prep_inputs(x, gamma, beta, Wq, bq, Wk, bk, Wv, bv, Wp, bp):
    """Build the 8 per-core input maps from full inputs."""
    B = x.shape[0]
    xf = np.ascontiguousarray(x.reshape(B, C, N)).astype(np.float32)
    bp_eff = (bp + Wp @ bv).astype(np.float32)

    gmask = np.zeros((CT, P, NG), np.float32)
    gmaskT = np.zeros((CT, NG + 1, P), np.float32)
    gf = gamma.astype(np.float32)
    bf = beta.astype(np.float32)
    for t in range(CT):
        for p in range(P):
            ch = t * P + p
            g = ch // GSZ
            gmask[t, p, g] = 1.0
            gmaskT[t, g, p] = gf[ch]
            gmaskT[t, NG, p] = bf[ch]

    shared = {
        "wq": np.ascontiguousarray(Wq.T).astype(_BF),
        "wk": np.ascontiguousarray(Wk.T).astype(_BF),
        "wv": np.ascontiguousarray(Wv.T).astype(_BF),
        "wp": np.ascontiguousarray(Wp.T).astype(_BF),
        "vecs": np.stack([bq, bk, gamma, beta], axis=1).astype(np.float32),
        "gmask": gmask,
        "gmaskT": gmaskT,
        "ones": np.ones((P, P), _BF),
    }
    in_maps = []
    for core in range(2 * B):
        b, h = divmod(core, 2)
        xb = xf[b]
        if h == 0:
            xp = xb
        else:
            xp = np.concatenate([xb[:, M:], xb[:, :M]], axis=1)
        m = dict(shared)
        m["xbf"] = np.ascontiguousarray(xp).astype(_BF)
        m["xres"] = np.ascontiguousarray(xp[:, :M]) + bp_eff[:, None]
        in_maps.append(m)
    return in_maps


def run(inputs, trace=False, **kw):
    x = np.asarray(inputs["x"], np.float32)
    B = x.shape[0]
    in_maps = _prep_inputs(**{k: np.asarray(v) for k, v in inputs.items()})
    nc = _get_nc()
    res = run_bass_kernel_spmd(nc, in_maps, core_ids=list(range(8)),
                               trace=trace, **kw)
    out = np.empty((B, C, N), np.float32)
    for core in range(2 * B):
        b, h = divmod(core, 2)
        out[b][:, h * M:(h + 1) * M] = res.results[core]["out"]
    return out.reshape(x.shape), res


def kernel(**inputs):
    out, _ = run(inputs, trace=False)
    return out


# revision 24
# speedup vs baseline: 1.0102x; 1.0034x over previous
"""Trainium2 Bass kernel for an AttnBlock (GroupNorm + spatial self-attention
+ projection + residual), distributed over 8 NeuronCores.

Sharding: core = (batch b, query-half h). b=4 batches x 2 halves = 8 cores.
Each core receives x[b] with its spatial columns rotated so that its own
query half occupies columns 0:2048 (attention is permutation-invariant over
key positions, so rotating the key/value axis consistently is exact).
No collectives needed: K/V are computed locally from the full (rotated) x[b].

Self-contained: hardcodes shapes (b=4, c=512, h=w=64).
"""
import numpy as np
import ml_dtypes

import bass_rust
import concourse.bass as bass
import concourse.mybir as mybir
from concourse import tile
from concourse.bass_utils import run_bass_kernel_spmd

f32 = mybir.dt.float32
bf16 = mybir.dt.bfloat16
AF = mybir.ActivationFunctionType

C = 512          # channels
N = 4096         # spatial positions (64*64)
M = 2048         # query positions per core (half)
P = 128          # partitions
CT = C // P      # 4 channel tiles
NT = N // P      # 32 n tiles
FB = 512         # free block (one PSUM bank of f32)
MB = M // FB     # 4 m-blocks per core
NG = 32          # groups
GSZ = C // NG    # 16 channels per group
EPS = 1e-6
RSCALE = 1.0 / np.sqrt(np.float32(C))   # attention scale
SSCALE = 1.0 / (GSZ * N)                # group-stat normalizer

_BF = ml_dtypes.bfloat16


def split_waits(nc, cap=1):
    """This walrus accepts one sync wait / one update per instruction; move
    extras onto adjacent same-engine NOPs (sequentially equivalent)."""
    for f in nc.m.functions:
        for bb in f.blocks:
            new_insts = []
            changed = False
            for inst in bb.instructions:
                si = inst.sync_info
                waits = list(si.on_wait) if si is not None else []
                ups = list(si.on_update) if si is not None else []
                if len(waits) > cap:
                    for ci in range(cap, len(waits), cap):
                        new_insts.append(mybir.InstNoOp(
                            name=f"{inst.name}-ws{ci}", engine=inst.engine,
                            ins=[], outs=[],
                            sync_info=bass_rust.SyncInfo(
                                on_wait=waits[ci:ci + cap], on_update=[])))
                    inst.sync_info = bass_rust.SyncInfo(
                        on_wait=waits[:cap], on_update=ups)
                    changed = True
                new_insts.append(inst)
                if len(ups) > 1:
                    inst.sync_info = bass_rust.SyncInfo(
                        on_wait=list(inst.sync_info.on_wait), on_update=ups[:1])
                    for ui in range(1, len(ups)):
                        new_insts.append(mybir.InstNoOp(
                            name=f"{inst.name}-us{ui}", engine=inst.engine,
                            ins=[], outs=[],
                            sync_info=bass_rust.SyncInfo(
                                on_wait=[], on_update=[ups[ui]])))
                    changed = True
            if changed:
                bb.instructions = new_insts


def build():
    nc = bass.Bass()

    xbf_e = nc.declare_dram_parameter("xbf", [C, N], bf16, isOutput=False)
    wq_e = nc.declare_dram_parameter("wq", [C, C], bf16, isOutput=False)
    wk_e = nc.declare_dram_parameter("wk", [C, C], bf16, isOutput=False)
    wv_e = nc.declare_dram_parameter("wv", [C, C], bf16, isOutput=False)
    wp_e = nc.declare_dram_parameter("wp", [C, C], bf16, isOutput=False)
    vecs_e = nc.declare_dram_parameter("vecs", [C, 4], f32, isOutput=False)
    gm_e = nc.declare_dram_parameter("gmask", [CT, P, NG], f32, isOutput=False)
    gmt_e = nc.declare_dram_parameter("gmaskT", [CT, NG + 1, P], f32, isOutput=False)
    ones_e = nc.declare_dram_parameter("ones", [P, P], bf16, isOutput=False)
    xres_e = nc.declare_dram_parameter("xres", [C, M], f32, isOutput=False)
    out_e = nc.declare_dram_parameter("out", [C, M], f32, isOutput=True)
    warm_e = nc.dram_tensor("warm_sink", [1, 4], f32)

    with tile.TileContext(nc) as tc:
        with (
            tc.tile_pool(name="const", bufs=1) as cp,
            tc.tile_pool(name="big", bufs=1) as bp,
            tc.tile_pool(name="small", bufs=1) as sp,
            tc.tile_pool(name="work", bufs=3) as wkp,
            tc.tile_pool(name="pmm", bufs=3, space="PSUM") as pmm,
            tc.tile_pool(name="pu", bufs=1, space="PSUM") as pu,
            tc.tile_pool(name="ps", bufs=1, space="PSUM") as psp,
        ):
            # ---- x in (bf16) first, chunked; stats overlap the DMA.
            # Constants go through gpsimd's queue so their issue cost doesn't
            # delay the critical xbf chunks on sync's queue. ----
            # Chunk sizes stagger so the DMA->stats pipeline fills early:
            # tile 0 arrives in quarters (first stats op starts ~3us sooner),
            # later tiles arrive whole while earlier stats are in flight.
            CHUNKS = [1, 1, 1, 1]
            xbf_t = [bp.tile([P, N], bf16, tag=f"xbf{i}", name=f"xbf{i}") for i in range(CT)]
            for i in range(CT):
                w = N // CHUNKS[i]
                for c in range(CHUNKS[i]):
                    nc.sync.dma_start(
                        xbf_t[i][:, c * w:(c + 1) * w],
                        xbf_e[i * P:(i + 1) * P, c * w:(c + 1) * w])

            vec_t = [cp.tile([P, 4], f32, tag=f"vec{i}", name=f"vec{i}") for i in range(CT)]
            gm_t = [cp.tile([P, NG], f32, tag=f"gm{i}", name=f"gm{i}") for i in range(CT)]
            gmt_t = [cp.tile([NG + 1, P], f32, tag=f"gmt{i}", name=f"gmt{i}") for i in range(CT)]
            for i in range(CT):
                sl = slice(i * P, (i + 1) * P)
                nc.sync.dma_start(vec_t[i][:], vecs_e[sl, :])
                nc.sync.dma_start(gm_t[i][:], gm_e[i, :, :])
                nc.sync.dma_start(gmt_t[i][:], gmt_e[i, :, :])
            ones_t = cp.tile([P, P], bf16, tag="ones", name="ones")
            nc.gpsimd.dma_start(ones_t[:], ones_e[:])
            wq_t = [cp.tile([P, C], bf16, tag=f"wq{i}", name=f"wq{i}") for i in range(CT)]
            wk_t = [cp.tile([P, C], bf16, tag=f"wk{i}", name=f"wk{i}") for i in range(CT)]
            wv_t = [cp.tile([P, C], bf16, tag=f"wv{i}", name=f"wv{i}") for i in range(CT)]
            wp_t = [cp.tile([P, C], bf16, tag=f"wp{i}", name=f"wp{i}") for i in range(CT)]
            for i in range(CT):
                nc.gpsimd.dma_start(wq_t[i][:], wq_e[i * P:(i + 1) * P, :])
                nc.gpsimd.dma_start(wk_t[i][:], wk_e[i * P:(i + 1) * P, :])
                nc.gpsimd.dma_start(wv_t[i][:], wv_e[i * P:(i + 1) * P, :])
                nc.gpsimd.dma_start(wp_t[i][:], wp_e[i * P:(i + 1) * P, :])
            bq_t = [vec_t[i][:, 0:1] for i in range(CT)]
            bk_t = [vec_t[i][:, 1:2] for i in range(CT)]
            gam_t = [vec_t[i][:, 2:3] for i in range(CT)]
            bet_t = [vec_t[i][:, 3:4] for i in range(CT)]

            # ---- group norm stats (whole-tile ops; the gather matmul
            # accumulates per tile so the 4-stage DMA->stats pipe overlaps) ----
            hn_t = [bp.tile([P, N], bf16, tag=f"hn{i}", name=f"hn{i}") for i in range(CT)]
            st2_t = [sp.tile([P, CHUNKS[i], 2], f32, tag=f"st2{i}",
                             name=f"st2{i}") for i in range(CT)]
            for i in range(CT):
                w = N // CHUNKS[i]
                for c in range(CHUNKS[i]):
                    csl = slice(c * w, (c + 1) * w)
                    # per-channel sum (DVE) and sum of squares (ACT accum); the
                    # ACT pass writes squares into hn as scratch (overwritten
                    # by the affine apply later).
                    nc.vector.tensor_reduce(
                        st2_t[i][:, c, 0:1], xbf_t[i][:, csl],
                        axis=mybir.AxisListType.X, op=mybir.AluOpType.add)
                    nc.scalar.activation(
                        hn_t[i][:, csl], xbf_t[i][:, csl], AF.Square,
                        accum_out=st2_t[i][:, c, 1:2])
            gps = psp.tile([NG, 2], f32, tag="s", name="s")
            ngath = sum(CHUNKS)
            gi = 0
            for i in range(CT):
                for c in range(CHUNKS[i]):
                    nc.tensor.matmul(
                        gps[:], gm_t[i][:], st2_t[i][:, c, :],
                        start=(gi == 0), stop=(gi == ngath - 1),
                        skip_group_check=True)
                    gi += 1
            # PE warm-up: HAM throttles PE to 1.2 GHz after idle; these dummy
            # matmuls fill the scalar-chain window so the projection matmuls
            # start at full clock. Token DMA keeps the chain live.
            WARMUP = 0
            if WARMUP:
                wps = pmm.tile([P, P], f32, tag="mm", name="warmps")
                for _ in range(WARMUP):
                    nc.tensor.matmul(wps[:], ones_t[:], ones_t[:],
                                     start=True, stop=True)
                wsb = sp.tile([1, 4], f32, tag="wsb", name="wsb")
                nc.vector.tensor_copy(wsb[:], wps[0:1, 0:4])
                nc.sync.dma_start(warm_e[:], wsb[:])
            # mean, rstd; gstat[:,1] transiently holds msq, then rstd
            gstat = sp.tile([NG, 2], f32, tag="gstat", name="gstat")   # [mean, rstd]
            mean = gstat[:, 0:1]
            nc.vector.tensor_scalar_mul(gstat[:, 0:2], gps[:, 0:2], SSCALE)
            m2 = sp.tile([NG, 1], f32, tag="m2", name="m2")
            nc.vector.tensor_mul(m2[:], mean, mean)
            varp = sp.tile([NG, 1], f32, tag="varp", name="varp")
            nc.vector.tensor_sub(varp[:], gstat[:, 1:2], m2[:])
            nc.vector.tensor_scalar_add(varp[:], varp[:], EPS)
            std = sp.tile([NG, 1], f32, tag="std", name="std")
            nc.scalar.activation(std[:], varp[:], AF.Sqrt)
            nc.vector.reciprocal(gstat[:, 1:2], std[:])

            # rhs33 = [[-mean*rstd, rstd]; [1, 0]]: with the gamma-scaled,
            # beta-extended maskT as lhsT, one matmul per tile produces
            # ex = [bias, scale] per channel (bias = beta - mean*gamma*rstd,
            # scale = gamma*rstd).
            rhs33 = sp.tile([NG + 1, 2], f32, tag="rhs33", name="rhs33")
            nc.gpsimd.memset(rhs33[NG:NG + 1, 0:1], 1.0)
            nc.gpsimd.memset(rhs33[NG:NG + 1, 1:2], 0.0)
            mr = sp.tile([NG, 1], f32, tag="mr", name="mr")
            nc.vector.tensor_mul(mr[:], gstat[:, 0:1], gstat[:, 1:2])
            nc.vector.tensor_scalar_mul(rhs33[0:NG, 0:1], mr[:], -1.0)
            nc.vector.tensor_copy(rhs33[0:NG, 1:2], gstat[:, 1:2])
            ab_t = []
            for i in range(CT):
                eps_p = pmm.tile([P, 2], f32, tag="mm", name="mm")
                nc.tensor.matmul(eps_p[:], gmt_t[i][:], rhs33[:],
                                 start=True, stop=True)
                ex = sp.tile([P, 2], f32, tag=f"ex{i}", name=f"ex{i}")
                nc.vector.tensor_copy(ex[:], eps_p[:])
                ab_t.append(ex)
            # chunked apply (512-wide) so downstream matmuls start early
            for c in range(N // FB):
                for i in range(CT):
                    csl = slice(c * FB, (c + 1) * FB)
                    nc.scalar.activation(
                        hn_t[i][:, csl], xbf_t[i][:, csl], AF.Identity,
                        bias=ab_t[i][:, 0:1], scale=ab_t[i][:, 1:2])

            # ---- projections ----
            q_t = [bp.tile([P, M], bf16, tag=f"q{i}", name=f"q{i}") for i in range(CT)]
            k_t = [bp.tile([P, N], bf16, tag=f"k{i}", name=f"k{i}") for i in range(CT)]
            vt_t = bp.tile([P, NT * C], bf16, tag="vt", name="vt")  # [n-tile stack | c]

            for ot in range(CT):
                for b in range(MB):
                    ps = pmm.tile([P, FB], f32, tag="mm", name="mm")
                    for kt in range(CT):
                        nc.tensor.matmul(
                            ps[:], wq_t[kt][:, ot * P:(ot + 1) * P],
                            hn_t[kt][:, b * FB:(b + 1) * FB],
                            start=(kt == 0), stop=(kt == CT - 1))
                    nc.vector.tensor_scalar_add(
                        q_t[ot][:, b * FB:(b + 1) * FB], ps[:], bq_t[ot])
            for ot in range(CT):
                for b in range(N // FB):
                    ps = pmm.tile([P, FB], f32, tag="mm", name="mm")
                    for kt in range(CT):
                        nc.tensor.matmul(
                            ps[:], wk_t[kt][:, ot * P:(ot + 1) * P],
                            hn_t[kt][:, b * FB:(b + 1) * FB],
                            start=(kt == 0), stop=(kt == CT - 1))
                    nc.vector.tensor_scalar_add(
                        k_t[ot][:, b * FB:(b + 1) * FB], ps[:], bk_t[ot])
            for nt in range(NT):
                ps = pmm.tile([P, C], f32, tag="mm", name="mm")
                for kt in range(CT):
                    nc.tensor.matmul(
                        ps[:], hn_t[kt][:, nt * P:(nt + 1) * P], wv_t[kt][:],
                        start=(kt == 0), stop=(kt == CT - 1))
                nc.vector.tensor_copy(vt_t[:, nt * C:(nt + 1) * C], ps[:])

            # ---- attention (per m-block) ----
            # Software-pipelined: exp consumption lags the score matmuls by
            # LAG n-tiles, and the previous m-block's tail (reciprocal,
            # normalize, projection, residual, store) is emitted a few
            # n-tiles into the next block so PE never waits on DVE.
            LAG = 4   # u-matmul consumption lag (n-tiles)
            SLAG = 1  # s-matmul lag: early so the reciprocal overlaps the
                      # final u-matmuls instead of serializing after them

            def consume_u(b, j, e_sb, u_ps):
                for ct in range(CT):
                    nc.tensor.matmul(
                        u_ps[ct][:],
                        vt_t[:, j * C + ct * P: j * C + (ct + 1) * P],
                        e_sb[:],
                        start=(j == 0), stop=(j == NT - 1),
                        skip_group_check=True)

            def consume_s(b, j, e_sb, s_ps):
                nc.tensor.matmul(
                    s_ps[:], ones_t[:], e_sb[:],
                    start=(j == 0), stop=(j == NT - 1), skip_group_check=True)

            def emit_tail(b, u_ps, s_ps, nsplit=1):
                # nsplit>1 shortens the serial reciprocal->normalize->project
                # chain; used for the final block where nothing hides it.
                HB = FB // nsplit
                for hb in range(nsplit):
                    hsl = slice(hb * HB, (hb + 1) * HB)
                    msl = slice(b * FB + hb * HB, b * FB + (hb + 1) * HB)
                    r_sb = wkp.tile([P, HB], f32, tag="r", name="r")
                    nc.vector.reciprocal(r_sb[:], s_ps[:, hsl])
                    u_sb = [wkp.tile([P, HB], bf16, tag=f"usb{ct}",
                                     name=f"usb{ct}") for ct in range(CT)]
                    for ct in range(CT):
                        nc.vector.tensor_mul(u_sb[ct][:], u_ps[ct][:, hsl],
                                             r_sb[:])
                    for ot in range(CT):
                        pp = pmm.tile([P, HB], f32, tag="mm", name="mm")
                        for kt in range(CT):
                            nc.tensor.matmul(
                                pp[:], wp_t[kt][:, ot * P:(ot + 1) * P],
                                u_sb[kt][:],
                                start=(kt == 0), stop=(kt == CT - 1))
                        xr = wkp.tile([P, HB], f32, tag="xr", name="xr")
                        nc.sync.dma_start(
                            xr[:], xres_e[ot * P:(ot + 1) * P, msl])
                        o_sb = wkp.tile([P, HB], f32, tag="o", name="o")
                        nc.vector.tensor_add(o_sb[:], pp[:], xr[:])
                        nc.sync.dma_start(
                            out_e[ot * P:(ot + 1) * P, msl], o_sb[:])

            prev_tail = None
            for b in range(MB):
                msl = slice(b * FB, (b + 1) * FB)
                u_ps = [pu.tile([P, FB], f32, tag=f"u{ct}", name=f"u{ct}")
                        for ct in range(CT)]
                s_ps = psp.tile([P, FB], f32, tag="s", name="s")
                es = []
                for nt in range(NT):
                    sc = pmm.tile([P, FB], f32, tag="mm", name="mm")
                    for kt in range(CT):
                        nc.tensor.matmul(
                            sc[:], k_t[kt][:, nt * P:(nt + 1) * P],
                            q_t[kt][:, msl],
                            start=(kt == 0), stop=(kt == CT - 1))
                    if nt >= SLAG:
                        consume_s(b, nt - SLAG, es[nt - SLAG], s_ps)
                    e_sb = wkp.tile([P, FB], bf16, tag="e", name="e", bufs=LAG + 2)
                    nc.scalar.activation(e_sb[:], sc[:], AF.Exp, scale=RSCALE)
                    es.append(e_sb)
                    if nt == LAG - 1 and prev_tail is not None:
                        emit_tail(*prev_tail)
                        prev_tail = None
                    if nt >= LAG:
                        consume_u(b, nt - LAG, es[nt - LAG], u_ps)
                for j in range(NT - SLAG, NT):
                    consume_s(b, j, es[j], s_ps)
                for j in range(NT - LAG, NT):
                    consume_u(b, j, es[j], u_ps)
                prev_tail = (b, u_ps, s_ps)
            emit_tail(*prev_tail)

    split_waits(nc)
    return nc


_NC_CACHE = None


def _get_nc():
    global _NC_CACHE
    if _NC_CACHE is None:
        _NC_CACHE = build()
    return _NC_CACHE


def _prep_inputs(x, gamma, beta, Wq, bq, Wk, bk, Wv, bv, Wp, bp):
    """Build the 8 per-core input maps from full inputs."""
    B = x.shape[0]
    xf = np.ascontiguousarray(x.reshape(B, C, N)).astype(np.float32)
    bp_eff = (bp + Wp @ bv).astype(np.float32)

    gmask = np.zeros((CT, P, NG), np.float32)
    gmaskT = np.zeros((CT, NG + 1, P), np.float32)
    gf = gamma.astype(np.float32)
    bf = beta.astype(np.float32)
    for t in range(CT):
        for p in range(P):
            ch = t * P + p
            g = ch // GSZ
            gmask[t, p, g] = 1.0
            gmaskT[t, g, p] = gf[ch]
            gmaskT[t, NG, p] = bf[ch]

    shared = {
        "wq": np.ascontiguousarray(Wq.T).astype(_BF),
        "wk": np.ascontiguousarray(Wk.T).astype(_BF),
        "wv": np.ascontiguousarray(Wv.T).astype(_BF),
        "wp": np.ascontiguousarray(Wp.T).astype(_BF),
        "vecs": np.stack([bq, bk, gamma, beta], axis=1).astype(np.float32),
        "gmask": gmask,
        "gmaskT": gmaskT,
        "ones": np.ones((P, P), _BF),
    }
    in_maps = []
    for core in range(2 * B):
        b, h = divmod(core, 2)
        xb = xf[b]
        if h == 0:
            xp = xb
        else:
            xp = np.concatenate([xb[:, M:], xb[:, :M]], axis=1)
        m = dict(shared)
        m["xbf"] = np.ascontiguousarray(xp).astype(_BF)
        m["xres"] = np.ascontiguousarray(xp[:, :M]) + bp_eff[:, None]
        in_maps.append(m)
    return in_maps


def run(inputs, trace=False, **kw):
    x = np.asarray(inputs["x"], np.float32)
    B = x.shape[0]
    in_maps = _prep_inputs(**{k: np.asarray(v) for k, v in inputs.items()})
    nc = _get_nc()
    res = run_bass_kernel_spmd(nc, in_maps, core_ids=list(range(8)),
                               trace=trace, **kw)
    out = np.empty((B, C, N), np.float32)
    for core in range(2 * B):
        b, h = divmod(core, 2)
        out[b][:, h * M:(h + 1) * M] = res.results[core]["out"]
    return out.reshape(x.shape), res


def kernel(**inputs):
    out, _ = run(inputs, trace=False)
    return out


# revision 30
# speedup vs baseline: 1.0322x; 1.0218x over previous
"""Trainium2 Bass kernel for an AttnBlock (GroupNorm + spatial self-attention
+ projection + residual), distributed over 8 NeuronCores.

Sharding: core = (batch b, query-half h). b=4 batches x 2 halves = 8 cores.
Each core receives x[b] with its spatial columns rotated so that its own
query half occupies columns 0:2048 (attention is permutation-invariant over
key positions, so rotating the key/value axis consistently is exact).
No collectives needed: K/V are computed locally from the full (rotated) x[b].

Self-contained: hardcodes shapes (b=4, c=512, h=w=64).
"""
import numpy as np
import ml_dtypes

import bass_rust
import concourse.bass as bass
import concourse.mybir as mybir
from concourse import tile
from concourse.bass_utils import run_bass_kernel_spmd

f32 = mybir.dt.float32
bf16 = mybir.dt.bfloat16
AF = mybir.ActivationFunctionType

C = 512          # channels
N = 4096         # spatial positions (64*64)
M = 2048         # query positions per core (half)
P = 128          # partitions
CT = C // P      # 4 channel tiles
NT = N // P      # 32 n tiles
FB = 512         # free block (one PSUM bank of f32)
MB = M // FB     # 4 m-blocks per core
NG = 32          # groups
GSZ = C // NG    # 16 channels per group
EPS = 1e-6
RSCALE = 1.0 / np.sqrt(np.float32(C))   # attention scale
SSCALE = 1.0 / (GSZ * N)                # group-stat normalizer

_BF = ml_dtypes.bfloat16


def split_waits(nc, cap=1):
    """This walrus accepts one sync wait / one update per instruction; move
    extras onto adjacent same-engine NOPs (sequentially equivalent)."""
    for f in nc.m.functions:
        for bb in f.blocks:
            new_insts = []
            changed = False
            for inst in bb.instructions:
                si = inst.sync_info
                waits = list(si.on_wait) if si is not None else []
                ups = list(si.on_update) if si is not None else []
                if len(waits) > cap:
                    for ci in range(cap, len(waits), cap):
                        new_insts.append(mybir.InstNoOp(
                            name=f"{inst.name}-ws{ci}", engine=inst.engine,
                            ins=[], outs=[],
                            sync_info=bass_rust.SyncInfo(
                                on_wait=waits[ci:ci + cap], on_update=[])))
                    inst.sync_info = bass_rust.SyncInfo(
                        on_wait=waits[:cap], on_update=ups)
                    changed = True
                new_insts.append(inst)
                if len(ups) > 1:
                    inst.sync_info = bass_rust.SyncInfo(
                        on_wait=list(inst.sync_info.on_wait), on_update=ups[:1])
                    for ui in range(1, len(ups)):
                        new_insts.append(mybir.InstNoOp(
                            name=f"{inst.name}-us{ui}", engine=inst.engine,
                            ins=[], outs=[],
                            sync_info=bass_rust.SyncInfo(
                                on_wait=[], on_update=[ups[ui]])))
                    changed = True
            if changed:
                bb.instructions = new_insts


def build():
    nc = bass.Bass()

    xbf_e = nc.declare_dram_parameter("xbf", [C, N], bf16, isOutput=False)
    wq_e = nc.declare_dram_parameter("wq", [C, C], bf16, isOutput=False)
    wv_e = nc.declare_dram_parameter("wv", [C, C], bf16, isOutput=False)
    wp_e = nc.declare_dram_parameter("wp", [C, C], bf16, isOutput=False)
    wtil_e = nc.declare_dram_parameter("wtil", [C, 1], bf16, isOutput=False)
    gm_e = nc.declare_dram_parameter("gmask", [CT, P, NG], f32, isOutput=False)
    gmt_e = nc.declare_dram_parameter("gmaskT", [CT, NG + 1, P], f32, isOutput=False)
    ones_e = nc.declare_dram_parameter("ones", [P, P], bf16, isOutput=False)
    xres_e = nc.declare_dram_parameter("xres", [C, M], f32, isOutput=False)
    out_e = nc.declare_dram_parameter("out", [C, M], f32, isOutput=True)
    warm_e = nc.dram_tensor("warm_sink", [1, 4], f32)

    with tile.TileContext(nc) as tc:
        with (
            tc.tile_pool(name="const", bufs=1) as cp,
            tc.tile_pool(name="big", bufs=1) as bp,
            tc.tile_pool(name="small", bufs=1) as sp,
            tc.tile_pool(name="work", bufs=3) as wkp,
            tc.tile_pool(name="pmm", bufs=3, space="PSUM") as pmm,
            tc.tile_pool(name="pu", bufs=1, space="PSUM") as pu,
            tc.tile_pool(name="ps", bufs=1, space="PSUM") as psp,
        ):
            # ---- x in (bf16) first, chunked; stats overlap the DMA.
            # Constants go through gpsimd's queue so their issue cost doesn't
            # delay the critical xbf chunks on sync's queue. ----
            # Chunk sizes stagger so the DMA->stats pipeline fills early:
            # tile 0 arrives in quarters (first stats op starts ~3us sooner),
            # later tiles arrive whole while earlier stats are in flight.
            CHUNKS = [1, 1, 1, 1]
            xbf_t = [bp.tile([P, N], bf16, tag=f"xbf{i}", name=f"xbf{i}") for i in range(CT)]
            for i in range(CT):
                w = N // CHUNKS[i]
                for c in range(CHUNKS[i]):
                    nc.sync.dma_start(
                        xbf_t[i][:, c * w:(c + 1) * w],
                        xbf_e[i * P:(i + 1) * P, c * w:(c + 1) * w])

            gm_t = [cp.tile([P, NG], f32, tag=f"gm{i}", name=f"gm{i}") for i in range(CT)]
            gmt_t = [cp.tile([NG + 1, P], f32, tag=f"gmt{i}", name=f"gmt{i}") for i in range(CT)]
            wtil_t = [cp.tile([P, 1], bf16, tag=f"wt{i}", name=f"wt{i}") for i in range(CT)]
            for i in range(CT):
                sl = slice(i * P, (i + 1) * P)
                nc.sync.dma_start(gm_t[i][:], gm_e[i, :, :])
                nc.sync.dma_start(gmt_t[i][:], gmt_e[i, :, :])
                nc.sync.dma_start(wtil_t[i][:], wtil_e[sl, :])
            ones_t = cp.tile([P, P], bf16, tag="ones", name="ones")
            nc.gpsimd.dma_start(ones_t[:], ones_e[:])
            wq_t = [cp.tile([P, C], bf16, tag=f"wq{i}", name=f"wq{i}") for i in range(CT)]
            wv_t = [cp.tile([P, C], bf16, tag=f"wv{i}", name=f"wv{i}") for i in range(CT)]
            wp_t = [cp.tile([P, C], bf16, tag=f"wp{i}", name=f"wp{i}") for i in range(CT)]
            for i in range(CT):
                nc.gpsimd.dma_start(wq_t[i][:], wq_e[i * P:(i + 1) * P, :])
                nc.gpsimd.dma_start(wv_t[i][:], wv_e[i * P:(i + 1) * P, :])
                nc.gpsimd.dma_start(wp_t[i][:], wp_e[i * P:(i + 1) * P, :])

            # ---- group norm stats (whole-tile ops; the gather matmul
            # accumulates per tile so the 4-stage DMA->stats pipe overlaps) ----
            hn_t = [bp.tile([P, N], bf16, tag=f"hn{i}", name=f"hn{i}") for i in range(CT)]
            st2_t = [sp.tile([P, CHUNKS[i], 2], f32, tag=f"st2{i}",
                             name=f"st2{i}") for i in range(CT)]
            for i in range(CT):
                w = N // CHUNKS[i]
                for c in range(CHUNKS[i]):
                    csl = slice(c * w, (c + 1) * w)
                    # per-channel sum (DVE) and sum of squares (ACT accum); the
                    # ACT pass writes squares into hn as scratch (overwritten
                    # by the affine apply later).
                    nc.vector.tensor_reduce(
                        st2_t[i][:, c, 0:1], xbf_t[i][:, csl],
                        axis=mybir.AxisListType.X, op=mybir.AluOpType.add)
                    nc.scalar.activation(
                        hn_t[i][:, csl], xbf_t[i][:, csl], AF.Square,
                        accum_out=st2_t[i][:, c, 1:2])
            gps = psp.tile([NG, 2], f32, tag="s", name="s")
            ngath = sum(CHUNKS)
            gi = 0
            for i in range(CT):
                for c in range(CHUNKS[i]):
                    nc.tensor.matmul(
                        gps[:], gm_t[i][:], st2_t[i][:, c, :],
                        start=(gi == 0), stop=(gi == ngath - 1),
                        skip_group_check=True)
                    gi += 1
            # PE warm-up: HAM throttles PE to 1.2 GHz after idle; these dummy
            # matmuls fill the scalar-chain window so the projection matmuls
            # start at full clock. Token DMA keeps the chain live.
            WARMUP = 0
            if WARMUP:
                wps = pmm.tile([P, P], f32, tag="mm", name="warmps")
                for _ in range(WARMUP):
                    nc.tensor.matmul(wps[:], ones_t[:], ones_t[:],
                                     start=True, stop=True)
                wsb = sp.tile([1, 4], f32, tag="wsb", name="wsb")
                nc.vector.tensor_copy(wsb[:], wps[0:1, 0:4])
                nc.sync.dma_start(warm_e[:], wsb[:])
            # mean, rstd; gstat[:,1] transiently holds msq, then rstd
            gstat = sp.tile([NG, 2], f32, tag="gstat", name="gstat")   # [mean, rstd]
            mean = gstat[:, 0:1]
            nc.vector.tensor_scalar_mul(gstat[:, 0:2], gps[:, 0:2], SSCALE)
            m2 = sp.tile([NG, 1], f32, tag="m2", name="m2")
            nc.vector.tensor_mul(m2[:], mean, mean)
            varp = sp.tile([NG, 1], f32, tag="varp", name="varp")
            nc.vector.tensor_sub(varp[:], gstat[:, 1:2], m2[:])
            nc.vector.tensor_scalar_add(varp[:], varp[:], EPS)
            std = sp.tile([NG, 1], f32, tag="std", name="std")
            nc.scalar.activation(std[:], varp[:], AF.Sqrt)
            nc.vector.reciprocal(gstat[:, 1:2], std[:])

            # rhs33 = [[-mean*rstd, rstd]; [1, 0]]: with the gamma-scaled,
            # beta-extended maskT as lhsT, one matmul per tile produces
            # ex = [bias, scale] per channel (bias = beta - mean*gamma*rstd,
            # scale = gamma*rstd).
            rhs33 = sp.tile([NG + 1, 2], f32, tag="rhs33", name="rhs33")
            nc.gpsimd.memset(rhs33[NG:NG + 1, 0:1], 1.0)
            nc.gpsimd.memset(rhs33[NG:NG + 1, 1:2], 0.0)
            mr = sp.tile([NG, 1], f32, tag="mr", name="mr")
            nc.vector.tensor_mul(mr[:], gstat[:, 0:1], gstat[:, 1:2])
            nc.vector.tensor_scalar_mul(rhs33[0:NG, 0:1], mr[:], -1.0)
            nc.vector.tensor_copy(rhs33[0:NG, 1:2], gstat[:, 1:2])
            ab_t = []
            for i in range(CT):
                eps_p = pmm.tile([P, 2], f32, tag="mm", name="mm")
                nc.tensor.matmul(eps_p[:], gmt_t[i][:], rhs33[:],
                                 start=True, stop=True)
                ex = sp.tile([P, 2], f32, tag=f"ex{i}", name=f"ex{i}")
                nc.vector.tensor_copy(ex[:], eps_p[:])
                ab_t.append(ex)
            # chunked apply (512-wide) so downstream matmuls start early;
            # split across DVE (tensor_scalar, two per-partition AP scalars)
            # and ACT so the first q-matmul's four applies run in parallel
            for c in range(N // FB):
                for i in range(CT):
                    csl = slice(c * FB, (c + 1) * FB)
                    if i % 2 == 0:
                        nc.vector.tensor_scalar(
                            hn_t[i][:, csl], xbf_t[i][:, csl],
                            ab_t[i][:, 1:2], ab_t[i][:, 0:1],
                            op0=mybir.AluOpType.mult, op1=mybir.AluOpType.add)
                    else:
                        nc.scalar.activation(
                            hn_t[i][:, csl], xbf_t[i][:, csl], AF.Identity,
                            bias=ab_t[i][:, 0:1], scale=ab_t[i][:, 1:2])

            # ---- projections ----
            # scoresT = z^T @ hn with z = H^T hn, H = Wk^T Wq (host-folded):
            # replaces separate q and k projections. The bq column term drops
            # by softmax shift-invariance; the bk row term is the per-n bias
            # g = (Wk^T bq * RSCALE)^T hn, applied via the exp's bias AP after
            # a DRAM round-trip reshapes it from (1, n) to (n-partition, nt).
            k_t = [bp.tile([P, N], bf16, tag=f"k{i}", name=f"k{i}") for i in range(CT)]
            vt_t = bp.tile([P, NT * C], bf16, tag="vt", name="vt")  # [n-tile stack | c]

            for ot in range(CT):
                for b in range(N // FB):
                    ps = pmm.tile([P, FB], f32, tag="mm", name="mm")
                    for kt in range(CT):
                        nc.tensor.matmul(
                            ps[:], wq_t[kt][:, ot * P:(ot + 1) * P],
                            hn_t[kt][:, b * FB:(b + 1) * FB],
                            start=(kt == 0), stop=(kt == CT - 1))
                    nc.vector.tensor_copy(
                        k_t[ot][:, b * FB:(b + 1) * FB], ps[:])
            g_sb = sp.tile([1, N], f32, tag="gsb", name="gsb")
            for b in range(N // FB):
                gp = pmm.tile([1, FB], f32, tag="mm", name="mm")
                for kt in range(CT):
                    nc.tensor.matmul(
                        gp[:], wtil_t[kt][:], hn_t[kt][:, b * FB:(b + 1) * FB],
                        start=(kt == 0), stop=(kt == CT - 1))
                nc.vector.tensor_copy(g_sb[:, b * FB:(b + 1) * FB], gp[:])
            with tc.tile_pool(name="dram", bufs=1, space="DRAM") as dpool:
                g_d = dpool.tile([1, N], f32, tag="gd", name="gd")
                nc.sync.dma_start(g_d[:], g_sb[:])
                g_t = sp.tile([P, NT], f32, tag="gt", name="gt")
                nc.sync.dma_start(
                    g_t[:], g_d[:].rearrange("a (j p) -> (a p) j", p=P))
            for nt in range(NT):
                ps = pmm.tile([P, C], f32, tag="mm", name="mm")
                for kt in range(CT):
                    nc.tensor.matmul(
                        ps[:], hn_t[kt][:, nt * P:(nt + 1) * P], wv_t[kt][:],
                        start=(kt == 0), stop=(kt == CT - 1))
                nc.vector.tensor_copy(vt_t[:, nt * C:(nt + 1) * C], ps[:])

            # ---- attention (per m-block) ----
            # Software-pipelined: exp consumption lags the score matmuls by
            # LAG n-tiles, and the previous m-block's tail (reciprocal,
            # normalize, projection, residual, store) is emitted a few
            # n-tiles into the next block so PE never waits on DVE.
            LAG = 4   # u-matmul consumption lag (n-tiles)
            SLAG = 1  # s-matmul lag: early so the reciprocal overlaps the
                      # final u-matmuls instead of serializing after them

            def consume_u(b, j, e_sb, u_ps):
                for ct in range(CT):
                    nc.tensor.matmul(
                        u_ps[ct][:],
                        vt_t[:, j * C + ct * P: j * C + (ct + 1) * P],
                        e_sb[:],
                        start=(j == 0), stop=(j == NT - 1),
                        skip_group_check=True)

            def consume_s(b, j, e_sb, s_ps):
                nc.tensor.matmul(
                    s_ps[:], ones_t[:], e_sb[:],
                    start=(j == 0), stop=(j == NT - 1), skip_group_check=True)

            def emit_tail(b, u_ps, s_ps, nsplit=1):
                # nsplit>1 shortens the serial reciprocal->normalize->project
                # chain; used for the final block where nothing hides it.
                HB = FB // nsplit
                for hb in range(nsplit):
                    hsl = slice(hb * HB, (hb + 1) * HB)
                    msl = slice(b * FB + hb * HB, b * FB + (hb + 1) * HB)
                    r_sb = wkp.tile([P, HB], f32, tag="r", name="r")
                    nc.vector.reciprocal(r_sb[:], s_ps[:, hsl])
                    u_sb = [wkp.tile([P, HB], bf16, tag=f"usb{ct}",
                                     name=f"usb{ct}") for ct in range(CT)]
                    for ct in range(CT):
                        nc.vector.tensor_mul(u_sb[ct][:], u_ps[ct][:, hsl],
                                             r_sb[:])
                    for ot in range(CT):
                        pp = pmm.tile([P, HB], f32, tag="mm", name="mm")
                        for kt in range(CT):
                            nc.tensor.matmul(
                                pp[:], wp_t[kt][:, ot * P:(ot + 1) * P],
                                u_sb[kt][:],
                                start=(kt == 0), stop=(kt == CT - 1))
                        xr = wkp.tile([P, HB], f32, tag="xr", name="xr")
                        nc.sync.dma_start(
                            xr[:], xres_e[ot * P:(ot + 1) * P, msl])
                        o_sb = wkp.tile([P, HB], f32, tag="o", name="o")
                        nc.vector.tensor_add(o_sb[:], pp[:], xr[:])
                        nc.sync.dma_start(
                            out_e[ot * P:(ot + 1) * P, msl], o_sb[:])

            prev_tail = None
            for b in range(MB):
                msl = slice(b * FB, (b + 1) * FB)
                u_ps = [pu.tile([P, FB], f32, tag=f"u{ct}", name=f"u{ct}")
                        for ct in range(CT)]
                s_ps = psp.tile([P, FB], f32, tag="s", name="s")
                es = []
                for nt in range(NT):
                    sc = pmm.tile([P, FB], f32, tag="mm", name="mm")
                    for kt in range(CT):
                        nc.tensor.matmul(
                            sc[:], k_t[kt][:, nt * P:(nt + 1) * P],
                            hn_t[kt][:, msl],
                            start=(kt == 0), stop=(kt == CT - 1))
                    if nt >= SLAG:
                        consume_s(b, nt - SLAG, es[nt - SLAG], s_ps)
                    e_sb = wkp.tile([P, FB], bf16, tag="e", name="e", bufs=LAG + 2)
                    nc.scalar.activation(e_sb[:], sc[:], AF.Exp, scale=RSCALE,
                                         bias=g_t[:, nt:nt + 1])
                    es.append(e_sb)
                    if nt == LAG - 1 and prev_tail is not None:
                        emit_tail(*prev_tail)
                        prev_tail = None
                    if nt >= LAG:
                        consume_u(b, nt - LAG, es[nt - LAG], u_ps)
                for j in range(NT - SLAG, NT):
                    consume_s(b, j, es[j], s_ps)
                for j in range(NT - LAG, NT):
                    consume_u(b, j, es[j], u_ps)
                prev_tail = (b, u_ps, s_ps)
            emit_tail(*prev_tail)

    split_waits(nc)
    return nc


_NC_CACHE = None


def _get_nc():
    global _NC_CACHE
    if _NC_CACHE is None:
        _NC_CACHE = build()
    return _NC_CACHE


def _prep_inputs(x, gamma, beta, Wq, bq, Wk, bk, Wv, bv, Wp, bp):
    """Build the 8 per-core input maps from full inputs."""
    B = x.shape[0]
    xf = np.ascontiguousarray(x.reshape(B, C, N)).astype(np.float32)
    bp_eff = (bp + Wp @ bv).astype(np.float32)

    gmask = np.zeros((CT, P, NG), np.float32)
    gmaskT = np.zeros((CT, NG + 1, P), np.float32)
    gf = gamma.astype(np.float32)
    bf = beta.astype(np.float32)
    for t in range(CT):
        for p in range(P):
            ch = t * P + p
            g = ch // GSZ
            gmask[t, p, g] = 1.0
            gmaskT[t, g, p] = gf[ch]
            gmaskT[t, NG, p] = bf[ch]

    H = (Wk.T @ Wq).astype(np.float32)
    wtil = (Wk.T @ bq * np.float32(RSCALE)).astype(np.float32)
    shared = {
        "wq": np.ascontiguousarray(H).astype(_BF),
        "wv": np.ascontiguousarray(Wv.T).astype(_BF),
        "wp": np.ascontiguousarray(Wp.T).astype(_BF),
        "wtil": wtil.reshape(C, 1).astype(_BF),
        "gmask": gmask,
        "gmaskT": gmaskT,
        "ones": np.ones((P, P), _BF),
    }
    in_maps = []
    for core in range(2 * B):
        b, h = divmod(core, 2)
        xb = xf[b]
        if h == 0:
            xp = xb
        else:
            xp = np.concatenate([xb[:, M:], xb[:, :M]], axis=1)
        m = dict(shared)
        m["xbf"] = np.ascontiguousarray(xp).astype(_BF)
        m["xres"] = np.ascontiguousarray(xp[:, :M]) + bp_eff[:, None]
        in_maps.append(m)
    return in_maps


def run(inputs, trace=False, **kw):
    x = np.asarray(inputs["x"], np.float32)
    B = x.shape[0]
    in_maps = _prep_inputs(**{k: np.asarray(v) for k, v in inputs.items()})
    nc = _get_nc()
    res = run_bass_kernel_spmd(nc, in_maps, core_ids=list(range(8)),
                               trace=trace, **kw)
    out = np.empty((B, C, N), np.float32)
    for core in range(2 * B):
        b, h = divmod(core, 2)
        out[b][:, h * M:(h + 1) * M] = res.results[core]["out"]
    return out.reshape(x.shape), res


def kernel(**inputs):
    out, _ = run(inputs, trace=False)
    return out


# revision 32
# speedup vs baseline: 1.0349x; 1.0026x over previous
"""Trainium2 Bass kernel for an AttnBlock (GroupNorm + spatial self-attention
+ projection + residual), distributed over 8 NeuronCores.

Sharding: core = (batch b, query-half h). b=4 batches x 2 halves = 8 cores.
Each core receives x[b] with its spatial columns rotated so that its own
query half occupies columns 0:2048 (attention is permutation-invariant over
key positions, so rotating the key/value axis consistently is exact).
No collectives needed: K/V are computed locally from the full (rotated) x[b].

Self-contained: hardcodes shapes (b=4, c=512, h=w=64).
"""
import numpy as np
import ml_dtypes

import bass_rust
import concourse.bass as bass
import concourse.mybir as mybir
from concourse import tile
from concourse.bass_utils import run_bass_kernel_spmd

f32 = mybir.dt.float32
bf16 = mybir.dt.bfloat16
AF = mybir.ActivationFunctionType

C = 512          # channels
N = 4096         # spatial positions (64*64)
M = 2048         # query positions per core (half)
P = 128          # partitions
CT = C // P      # 4 channel tiles
NT = N // P      # 32 n tiles
FB = 512         # free block (one PSUM bank of f32)
MB = M // FB     # 4 m-blocks per core
NG = 32          # groups
GSZ = C // NG    # 16 channels per group
EPS = 1e-6
RSCALE = 1.0 / np.sqrt(np.float32(C))   # attention scale
SSCALE = 1.0 / (GSZ * N)                # group-stat normalizer

_BF = ml_dtypes.bfloat16


def split_waits(nc, cap=1):
    """This walrus accepts one sync wait / one update per instruction; move
    extras onto adjacent same-engine NOPs (sequentially equivalent)."""
    for f in nc.m.functions:
        for bb in f.blocks:
            new_insts = []
            changed = False
            for inst in bb.instructions:
                si = inst.sync_info
                waits = list(si.on_wait) if si is not None else []
                ups = list(si.on_update) if si is not None else []
                if len(waits) > cap:
                    for ci in range(cap, len(waits), cap):
                        new_insts.append(mybir.InstNoOp(
                            name=f"{inst.name}-ws{ci}", engine=inst.engine,
                            ins=[], outs=[],
                            sync_info=bass_rust.SyncInfo(
                                on_wait=waits[ci:ci + cap], on_update=[])))
                    inst.sync_info = bass_rust.SyncInfo(
                        on_wait=waits[:cap], on_update=ups)
                    changed = True
                new_insts.append(inst)
                if len(ups) > 1:
                    inst.sync_info = bass_rust.SyncInfo(
                        on_wait=list(inst.sync_info.on_wait), on_update=ups[:1])
                    for ui in range(1, len(ups)):
                        new_insts.append(mybir.InstNoOp(
                            name=f"{inst.name}-us{ui}", engine=inst.engine,
                            ins=[], outs=[],
                            sync_info=bass_rust.SyncInfo(
                                on_wait=[], on_update=[ups[ui]])))
                    changed = True
            if changed:
                bb.instructions = new_insts


def build():
    nc = bass.Bass()

    xbf_e = nc.declare_dram_parameter("xbf", [C, N], bf16, isOutput=False)
    wq_e = nc.declare_dram_parameter("wq", [C, C], bf16, isOutput=False)
    wv_e = nc.declare_dram_parameter("wv", [C, C], bf16, isOutput=False)
    wp_e = nc.declare_dram_parameter("wp", [C, C], bf16, isOutput=False)
    wtil_e = nc.declare_dram_parameter("wtil", [C, 1], bf16, isOutput=False)
    gm_e = nc.declare_dram_parameter("gmask", [CT, P, NG], f32, isOutput=False)
    gmt_e = nc.declare_dram_parameter("gmaskT", [CT, NG + 1, P], f32, isOutput=False)
    ones_e = nc.declare_dram_parameter("ones", [P, P], bf16, isOutput=False)
    xres_e = nc.declare_dram_parameter("xres", [C, M], f32, isOutput=False)
    out_e = nc.declare_dram_parameter("out", [C, M], f32, isOutput=True)
    warm_e = nc.dram_tensor("warm_sink", [1, 4], f32)

    with tile.TileContext(nc) as tc:
        with (
            tc.tile_pool(name="const", bufs=1) as cp,
            tc.tile_pool(name="big", bufs=1) as bp,
            tc.tile_pool(name="small", bufs=1) as sp,
            tc.tile_pool(name="work", bufs=3) as wkp,
            tc.tile_pool(name="pmm", bufs=3, space="PSUM") as pmm,
            tc.tile_pool(name="pu", bufs=1, space="PSUM") as pu,
            tc.tile_pool(name="ps", bufs=1, space="PSUM") as psp,
        ):
            # ---- x in (bf16) first, chunked; stats overlap the DMA.
            # Constants go through gpsimd's queue so their issue cost doesn't
            # delay the critical xbf chunks on sync's queue. ----
            # Chunk sizes stagger so the DMA->stats pipeline fills early:
            # tile 0 arrives in quarters (first stats op starts ~3us sooner),
            # later tiles arrive whole while earlier stats are in flight.
            CHUNKS = [1, 1, 1, 1]
            xbf_t = [bp.tile([P, N], bf16, tag=f"xbf{i}", name=f"xbf{i}") for i in range(CT)]
            for i in range(CT):
                w = N // CHUNKS[i]
                for c in range(CHUNKS[i]):
                    nc.sync.dma_start(
                        xbf_t[i][:, c * w:(c + 1) * w],
                        xbf_e[i * P:(i + 1) * P, c * w:(c + 1) * w])

            gm_t = [cp.tile([P, NG], f32, tag=f"gm{i}", name=f"gm{i}") for i in range(CT)]
            gmt_t = [cp.tile([NG + 1, P], f32, tag=f"gmt{i}", name=f"gmt{i}") for i in range(CT)]
            wtil_t = [cp.tile([P, 1], bf16, tag=f"wt{i}", name=f"wt{i}") for i in range(CT)]
            for i in range(CT):
                sl = slice(i * P, (i + 1) * P)
                nc.sync.dma_start(gm_t[i][:], gm_e[i, :, :])
                nc.sync.dma_start(gmt_t[i][:], gmt_e[i, :, :])
                nc.sync.dma_start(wtil_t[i][:], wtil_e[sl, :])
            ones_t = cp.tile([P, P], bf16, tag="ones", name="ones")
            nc.gpsimd.dma_start(ones_t[:], ones_e[:])
            wq_t = [cp.tile([P, C], bf16, tag=f"wq{i}", name=f"wq{i}") for i in range(CT)]
            wv_t = [cp.tile([P, C], bf16, tag=f"wv{i}", name=f"wv{i}") for i in range(CT)]
            wp_t = [cp.tile([P, C], bf16, tag=f"wp{i}", name=f"wp{i}") for i in range(CT)]
            for i in range(CT):
                nc.gpsimd.dma_start(wq_t[i][:], wq_e[i * P:(i + 1) * P, :])
                nc.gpsimd.dma_start(wv_t[i][:], wv_e[i * P:(i + 1) * P, :])
                nc.gpsimd.dma_start(wp_t[i][:], wp_e[i * P:(i + 1) * P, :])

            # ---- group norm stats (whole-tile ops; the gather matmul
            # accumulates per tile so the 4-stage DMA->stats pipe overlaps) ----
            hn_t = [bp.tile([P, N], bf16, tag=f"hn{i}", name=f"hn{i}") for i in range(CT)]
            st2_t = [sp.tile([P, CHUNKS[i], 2], f32, tag=f"st2{i}",
                             name=f"st2{i}") for i in range(CT)]
            for i in range(CT):
                w = N // CHUNKS[i]
                for c in range(CHUNKS[i]):
                    csl = slice(c * w, (c + 1) * w)
                    # per-channel sum (DVE) and sum of squares (ACT accum); the
                    # ACT pass writes squares into hn as scratch (overwritten
                    # by the affine apply later).
                    nc.vector.tensor_reduce(
                        st2_t[i][:, c, 0:1], xbf_t[i][:, csl],
                        axis=mybir.AxisListType.X, op=mybir.AluOpType.add)
                    nc.scalar.activation(
                        hn_t[i][:, csl], xbf_t[i][:, csl], AF.Square,
                        accum_out=st2_t[i][:, c, 1:2])
            gps = psp.tile([NG, 2], f32, tag="s", name="s")
            ngath = sum(CHUNKS)
            gi = 0
            for i in range(CT):
                for c in range(CHUNKS[i]):
                    nc.tensor.matmul(
                        gps[:], gm_t[i][:], st2_t[i][:, c, :],
                        start=(gi == 0), stop=(gi == ngath - 1),
                        skip_group_check=True)
                    gi += 1
            # PE warm-up: HAM throttles PE to 1.2 GHz after idle; these dummy
            # matmuls fill the scalar-chain window so the projection matmuls
            # start at full clock. Token DMA keeps the chain live.
            WARMUP = 0
            if WARMUP:
                wps = pmm.tile([P, P], f32, tag="mm", name="warmps")
                for _ in range(WARMUP):
                    nc.tensor.matmul(wps[:], ones_t[:], ones_t[:],
                                     start=True, stop=True)
                wsb = sp.tile([1, 4], f32, tag="wsb", name="wsb")
                nc.vector.tensor_copy(wsb[:], wps[0:1, 0:4])
                nc.sync.dma_start(warm_e[:], wsb[:])
            # mean, rstd; gstat[:,1] transiently holds msq, then rstd
            gstat = sp.tile([NG, 2], f32, tag="gstat", name="gstat")   # [mean, rstd]
            mean = gstat[:, 0:1]
            nc.vector.tensor_scalar_mul(gstat[:, 0:2], gps[:, 0:2], SSCALE)
            m2 = sp.tile([NG, 1], f32, tag="m2", name="m2")
            nc.vector.tensor_mul(m2[:], mean, mean)
            varp = sp.tile([NG, 1], f32, tag="varp", name="varp")
            nc.vector.tensor_sub(varp[:], gstat[:, 1:2], m2[:])
            nc.vector.tensor_scalar_add(varp[:], varp[:], EPS)
            std = sp.tile([NG, 1], f32, tag="std", name="std")
            nc.scalar.activation(std[:], varp[:], AF.Sqrt)
            nc.vector.reciprocal(gstat[:, 1:2], std[:])

            # rhs33 = [[-mean*rstd, rstd]; [1, 0]]: with the gamma-scaled,
            # beta-extended maskT as lhsT, one matmul per tile produces
            # ex = [bias, scale] per channel (bias = beta - mean*gamma*rstd,
            # scale = gamma*rstd).
            rhs33 = sp.tile([NG + 1, 2], f32, tag="rhs33", name="rhs33")
            nc.gpsimd.memset(rhs33[NG:NG + 1, 0:1], 1.0)
            nc.gpsimd.memset(rhs33[NG:NG + 1, 1:2], 0.0)
            mr = sp.tile([NG, 1], f32, tag="mr", name="mr")
            nc.vector.tensor_mul(mr[:], gstat[:, 0:1], gstat[:, 1:2])
            nc.vector.tensor_scalar_mul(rhs33[0:NG, 0:1], mr[:], -1.0)
            nc.vector.tensor_copy(rhs33[0:NG, 1:2], gstat[:, 1:2])
            ab_t = []
            for i in range(CT):
                eps_p = pmm.tile([P, 2], f32, tag="mm", name="mm")
                nc.tensor.matmul(eps_p[:], gmt_t[i][:], rhs33[:],
                                 start=True, stop=True)
                ex = sp.tile([P, 2], f32, tag=f"ex{i}", name=f"ex{i}")
                nc.vector.tensor_copy(ex[:], eps_p[:])
                ab_t.append(ex)
            # chunked apply (512-wide) so downstream matmuls start early;
            # split across DVE (tensor_scalar, two per-partition AP scalars)
            # and ACT so the first q-matmul's four applies run in parallel
            for c in range(N // FB):
                for i in range(CT):
                    csl = slice(c * FB, (c + 1) * FB)
                    if i % 2 == 0:
                        nc.vector.tensor_scalar(
                            hn_t[i][:, csl], xbf_t[i][:, csl],
                            ab_t[i][:, 1:2], ab_t[i][:, 0:1],
                            op0=mybir.AluOpType.mult, op1=mybir.AluOpType.add)
                    else:
                        nc.scalar.activation(
                            hn_t[i][:, csl], xbf_t[i][:, csl], AF.Identity,
                            bias=ab_t[i][:, 0:1], scale=ab_t[i][:, 1:2])

            # ---- projections ----
            # scoresT = z^T @ hn with z = H^T hn, H = Wk^T Wq (host-folded):
            # replaces separate q and k projections. The bq column term drops
            # by softmax shift-invariance; the bk row term is the per-n bias
            # g = (Wk^T bq * RSCALE)^T hn, applied via the exp's bias AP after
            # a DRAM round-trip reshapes it from (1, n) to (n-partition, nt).
            k_t = [bp.tile([P, N], bf16, tag=f"k{i}", name=f"k{i}") for i in range(CT)]
            vt_t = bp.tile([P, NT * C], bf16, tag="vt", name="vt")  # [n-tile stack | c]

            for ot in range(CT):
                for b in range(N // FB):
                    ps = pmm.tile([P, FB], f32, tag="mm", name="mm")
                    for kt in range(CT):
                        nc.tensor.matmul(
                            ps[:], wq_t[kt][:, ot * P:(ot + 1) * P],
                            hn_t[kt][:, b * FB:(b + 1) * FB],
                            start=(kt == 0), stop=(kt == CT - 1))
                    nc.vector.tensor_copy(
                        k_t[ot][:, b * FB:(b + 1) * FB], ps[:])
            g_sb = sp.tile([1, N], f32, tag="gsb", name="gsb")
            for b in range(N // FB):
                gp = pmm.tile([1, FB], f32, tag="mm", name="mm")
                for kt in range(CT):
                    nc.tensor.matmul(
                        gp[:], wtil_t[kt][:], hn_t[kt][:, b * FB:(b + 1) * FB],
                        start=(kt == 0), stop=(kt == CT - 1))
                nc.vector.tensor_copy(g_sb[:, b * FB:(b + 1) * FB], gp[:])
            with tc.tile_pool(name="dram", bufs=1, space="DRAM") as dpool:
                g_d = dpool.tile([1, N], f32, tag="gd", name="gd")
                nc.sync.dma_start(g_d[:], g_sb[:])
                g_t = sp.tile([P, NT], f32, tag="gt", name="gt")
                nc.sync.dma_start(
                    g_t[:], g_d[:].rearrange("a (j p) -> (a p) j", p=P))
            for nt in range(NT):
                ps = pmm.tile([P, C], f32, tag="mm", name="mm")
                for kt in range(CT):
                    nc.tensor.matmul(
                        ps[:], hn_t[kt][:, nt * P:(nt + 1) * P], wv_t[kt][:],
                        start=(kt == 0), stop=(kt == CT - 1))
                nc.vector.tensor_copy(vt_t[:, nt * C:(nt + 1) * C], ps[:])

            # ---- attention (per m-block) ----
            # Software-pipelined: exp consumption lags the score matmuls by
            # LAG n-tiles, and the previous m-block's tail (reciprocal,
            # normalize, projection, residual, store) is emitted a few
            # n-tiles into the next block so PE never waits on DVE.
            LAG = 4   # u-matmul consumption lag (n-tiles)
            SLAG = 1  # s-matmul lag: early so the reciprocal overlaps the
                      # final u-matmuls instead of serializing after them

            def consume_u(b, j, e_sb, u_ps):
                for ct in range(CT):
                    nc.tensor.matmul(
                        u_ps[ct][:],
                        vt_t[:, j * C + ct * P: j * C + (ct + 1) * P],
                        e_sb[:],
                        start=(j == 0), stop=(j == NT - 1),
                        skip_group_check=True)

            def consume_s(b, j, e_sb, s_ps):
                nc.tensor.matmul(
                    s_ps[:], ones_t[:], e_sb[:],
                    start=(j == 0), stop=(j == NT - 1), skip_group_check=True)

            def emit_tail(b, u_ps, s_ps, nsplit=1):
                # nsplit>1 shortens the serial reciprocal->normalize->project
                # chain; used for the final block where nothing hides it.
                HB = FB // nsplit
                for hb in range(nsplit):
                    hsl = slice(hb * HB, (hb + 1) * HB)
                    msl = slice(b * FB + hb * HB, b * FB + (hb + 1) * HB)
                    r_sb = wkp.tile([P, HB], f32, tag="r", name="r")
                    nc.vector.reciprocal(r_sb[:], s_ps[:, hsl])
                    u_sb = [wkp.tile([P, HB], bf16, tag=f"usb{ct}",
                                     name=f"usb{ct}") for ct in range(CT)]
                    for ct in range(CT):
                        nc.vector.tensor_mul(u_sb[ct][:], u_ps[ct][:, hsl],
                                             r_sb[:])
                    for ot in range(CT):
                        pp = pmm.tile([P, HB], f32, tag="mm", name="mm")
                        for kt in range(CT):
                            nc.tensor.matmul(
                                pp[:], wp_t[kt][:, ot * P:(ot + 1) * P],
                                u_sb[kt][:],
                                start=(kt == 0), stop=(kt == CT - 1))
                        xr = wkp.tile([P, HB], f32, tag="xr", name="xr")
                        nc.sync.dma_start(
                            xr[:], xres_e[ot * P:(ot + 1) * P, msl])
                        o_sb = wkp.tile([P, HB], f32, tag="o", name="o")
                        nc.vector.tensor_add(o_sb[:], pp[:], xr[:])
                        nc.sync.dma_start(
                            out_e[ot * P:(ot + 1) * P, msl], o_sb[:])

            prev_tail = None
            for b in range(MB):
                msl = slice(b * FB, (b + 1) * FB)
                u_ps = [pu.tile([P, FB], f32, tag=f"u{ct}", name=f"u{ct}")
                        for ct in range(CT)]
                s_ps = psp.tile([P, FB], f32, tag="s", name="s")
                es = []
                for nt in range(NT):
                    sc = pmm.tile([P, FB], f32, tag="mm", name="mm")
                    for kt in range(CT):
                        nc.tensor.matmul(
                            sc[:], k_t[kt][:, nt * P:(nt + 1) * P],
                            hn_t[kt][:, msl],
                            start=(kt == 0), stop=(kt == CT - 1))
                    if nt >= SLAG:
                        consume_s(b, nt - SLAG, es[nt - SLAG], s_ps)
                    e_sb = wkp.tile([P, FB], bf16, tag="e", name="e", bufs=LAG + 2)
                    nc.scalar.activation(e_sb[:], sc[:], AF.Exp, scale=RSCALE,
                                         bias=g_t[:, nt:nt + 1])
                    es.append(e_sb)
                    if nt == LAG - 1 and prev_tail is not None:
                        emit_tail(*prev_tail)
                        prev_tail = None
                    if nt >= LAG:
                        consume_u(b, nt - LAG, es[nt - LAG], u_ps)
                for j in range(NT - SLAG, NT):
                    consume_s(b, j, es[j], s_ps)
                for j in range(NT - LAG, NT):
                    consume_u(b, j, es[j], u_ps)
                prev_tail = (b, u_ps, s_ps)
            emit_tail(*prev_tail)

    split_waits(nc)
    return nc


_NC_CACHE = None


def _get_nc():
    global _NC_CACHE
    if _NC_CACHE is None:
        _NC_CACHE = build()
    return _NC_CACHE


def _prep_inputs(x, gamma, beta, Wq, bq, Wk, bk, Wv, bv, Wp, bp):
    """Build the 8 per-core input maps from full inputs."""
    B = x.shape[0]
    xf = np.ascontiguousarray(x.reshape(B, C, N)).astype(np.float32)
    bp_eff = (bp + Wp @ bv).astype(np.float32)

    gmask = np.zeros((CT, P, NG), np.float32)
    gmaskT = np.zeros((CT, NG + 1, P), np.float32)
    gf = gamma.astype(np.float32)
    bf = beta.astype(np.float32)
    for t in range(CT):
        for p in range(P):
            ch = t * P + p
            g = ch // GSZ
            gmask[t, p, g] = 1.0
            gmaskT[t, g, p] = gf[ch]
            gmaskT[t, NG, p] = bf[ch]

    H = (Wk.T @ Wq).astype(np.float32)
    wtil = (Wk.T @ bq * np.float32(RSCALE)).astype(np.float32)
    shared = {
        "wq": np.ascontiguousarray(H).astype(_BF),
        "wv": np.ascontiguousarray(Wv.T).astype(_BF),
        "wp": np.ascontiguousarray(Wp.T).astype(_BF),
        "wtil": wtil.reshape(C, 1).astype(_BF),
        "gmask": gmask,
        "gmaskT": gmaskT,
        "ones": np.ones((P, P), _BF),
    }
    in_maps = []
    for core in range(2 * B):
        b, h = divmod(core, 2)
        xb = xf[b]
        if h == 0:
            xp = xb
        else:
            xp = np.concatenate([xb[:, M:], xb[:, :M]], axis=1)
        m = dict(shared)
        m["xbf"] = np.ascontiguousarray(xp).astype(_BF)
        m["xres"] = np.ascontiguousarray(xp[:, :M]) + bp_eff[:, None]
        in_maps.append(m)
    return in_maps


def run(inputs, trace=False, **kw):
    x = np.asarray(inputs["x"], np.float32)
    B = x.shape[0]
    in_maps = _prep_inputs(**{k: np.asarray(v) for k, v in inputs.items()})
    nc = _get_nc()
    res = run_bass_kernel_spmd(nc, in_maps, core_ids=list(range(8)),
                               trace=trace, **kw)
    out = np.empty((B, C, N), np.float32)
    for core in range(2 * B):
        b, h = divmod(core, 2)
        out[b][:, h * M:(h + 1) * M] = res.results[core]["out"]
    return out.reshape(x.shape), res


def kernel(**inputs):
    out, _ = run(inputs, trace=False)
    return out
